# revision 27
# baseline (speedup 1.0000x reference)
"""Devign GGNN model on 8 Trainium2 NeuronCores (Bass/Tile) — v2.

Strategy (data-parallel over dst-node shards, aggregate-first message
passing, fp8 message path):
  - 8 cores, core c owns nodes [c*4096, (c+1)*4096) = graphs [c*16, +16).
  - Node state hT kept transposed on-chip ([256 feat, 4096 nodes] bf16)
    plus an fp8 kc-interleaved copy h8 [128, 2, 4096] for DoubleRow GRU.
  - Per GGNN step:
      1. AllGather the raw fp8 node states: in_cc [4096, 256] fp8 ->
         out_cc [32768, 256] fp8 (8x less wire traffic than gathering the
         per-etype transformed table in bf16).
      2. Edges (sorted by (dst-window, etype), chunked into 128-edge
         slots) are processed per 2-window group: indirect-DMA row gather
         of h[src] (fp8 256B rows, 4 SWDGE queues round-robin), then a
         one-hot scatter matmul agg_t += chunk^T @ S per (window, etype)
         using fp8 DoubleRow (2 chunks per matmul), S resident in SBUF.
      3. Transform after aggregation: aT[:, w] = sum_t W_msg[t]^T @ agg_t
         (fp8 DoubleRow) — 4x fewer transform FLOPs than transforming
         every node by every etype.
      4. GRU: all six gate matmuls as fp8 DoubleRow (contraction 256 in
         one instruction); gate pointwise + state update in bf16/f32.
      5. New h rows for the next AllGather produced by PE transpose +
         fp8 cast per 128-node chunk.
  - Readout (conv1d/maxpool stacks + gated sum) evaluated batched over
    all 16 graphs at full width with tap-shifted matmuls; invalid
    cross-graph columns masked before the final per-graph reduction.

All index/one-hot/weight-layout preprocessing is done on the host at
kernel() time and baked into the compiled program + per-core inputs.
"""
import sys
import numpy as np

for _p in ("/opt/trn_rl_repo",):
    if _p not in sys.path:
        sys.path.insert(0, _p)

import ml_dtypes

import concourse.bass as bass
import concourse.mybir as mybir
import concourse.tile as tile
from concourse import bacc
from concourse.bass_utils import run_bass_kernel_spmd

BF16 = ml_dtypes.bfloat16
FP8 = ml_dtypes.float8_e4m3
F32 = np.float32

NCORES = 8
NN = 32768          # total nodes
IN_DIM = 128
OUT = 256
NT = 4              # edge types
NSTEPS = 8
NGRAPH = 128
NPC = NN // NCORES  # nodes per core = 4096
WIN = 128           # dst window size
NWIN = NPC // WIN   # 32 windows per core
GW = 2              # windows per group / GRU block of 256 nodes
NGRP = NWIN // GW   # 16 groups
GPC = NGRAPH // NCORES  # graphs per core = 16
LG = 256            # nodes per graph
CONCAT = IN_DIM + OUT  # 384
CALLCH = 8          # chunks per dma_gather call (1024 rows)
NQ = 4              # SWDGE queues

bf = mybir.dt.bfloat16
f32 = mybir.dt.float32
fp8 = mybir.dt.float8e4
i16 = mybir.dt.int16
AF = mybir.ActivationFunctionType
ALU = mybir.AluOpType
DR = mybir.MatmulPerfMode.DoubleRow


# ---------------------------------------------------------------------------
# chunk layout shared between host packer and device builder
# ---------------------------------------------------------------------------
def _chunk_plan(CB):
    """CB: [NWIN][NT] chunk counts. Returns (boff[w][t] global chunk offset,
    SCBW[w], TOTCH, calls: list over grp of list of (c0, c1, coloff))."""
    CB = [[int(x) for x in row] for row in CB]
    boff = []
    tot = 0
    SCBW = []
    for w in range(NWIN):
        row = []
        for t in range(NT):
            row.append(tot)
            tot += CB[w][t]
        boff.append(row)
        SCBW.append(sum(CB[w]))
    TOTCH = tot
    # Gather calls aligned to (window, etype-pair) buckets so each call's
    # data is an independent tile the scatter can start on immediately.
    calls = []
    coloff = 0
    for g in range(NGRP):
        grp_calls = []
        for wi in range(GW):
            w = g * GW + wi
            for tp in range(2):
                c0 = boff[w][tp * 2]
                c1 = c0 + CB[w][tp * 2] + CB[w][tp * 2 + 1]
                if c1 - c0 > CALLCH:  # split oversized pair per etype
                    cm = c0 + CB[w][tp * 2]
                    grp_calls.append((c0, cm, coloff))
                    coloff += (cm - c0) * 128 // 16
                    grp_calls.append((cm, c1, coloff))
                    coloff += (c1 - cm) * 128 // 16
                else:
                    grp_calls.append((c0, c1, coloff))
                    coloff += (c1 - c0) * 128 // 16
        calls.append(grp_calls)
    return boff, SCBW, TOTCH, calls, coloff


# ---------------------------------------------------------------------------
# weight/bias image layout (bf16 conv image)
# ---------------------------------------------------------------------------
class WLayout:
    def __init__(self):
        self.col = 0
        self.off = {}

    def alloc(self, name, width):
        self.off[name] = self.col
        self.col += width
        return self.off[name]


def _make_wlayout():
    wl = WLayout()
    for tap in range(3):
        for kc in range(2):
            for mo in range(2):
                wl.alloc(f"c1_{tap}_{kc}_{mo}", 128)
    for kc in range(2):
        for mo in range(2):
            wl.alloc(f"c2_{kc}_{mo}", 128)
    for tap in range(3):
        for kc in range(3):
            for mo in range(3):
                wl.alloc(f"cc1_{tap}_{kc}_{mo}", 128)
    for kc in range(3):
        for mo in range(3):
            wl.alloc(f"cc2_{kc}_{mo}", 128)
    for kc in range(2):
        wl.alloc(f"wy_{kc}", 1)
    for kc in range(3):
        wl.alloc(f"wz_{kc}", 1)
    return wl


def _make_blayout():
    bl = WLayout()
    for name in ("br", "bz", "big", "bhg"):
        bl.alloc(name, 2)       # [256] as two [128] cols
    bl.alloc("c1b", 2)
    bl.alloc("c2b", 2)
    bl.alloc("cc1b", 3)
    bl.alloc("cc2b", 3)
    bl.alloc("by", 1)
    bl.alloc("bz_", 1)
    return bl


def _pack_weights(wl, conv1_w, conv2_w, convc1_w, convc2_w, wy, wz):
    img = np.zeros((128, wl.col), np.float32)

    def put(name, block):
        o = wl.off[name]
        img[:, o:o + block.shape[1]] = block

    for tap in range(3):
        w_t = conv1_w[:, :, tap].T  # [i, o]
        for kc in range(2):
            for mo in range(2):
                put(f"c1_{tap}_{kc}_{mo}", w_t[kc * 128:(kc + 1) * 128, mo * 128:(mo + 1) * 128])
    w2 = conv2_w[:, :, 0].T
    for kc in range(2):
        for mo in range(2):
            put(f"c2_{kc}_{mo}", w2[kc * 128:(kc + 1) * 128, mo * 128:(mo + 1) * 128])
    for tap in range(3):
        w_t = convc1_w[:, :, tap].T
        for kc in range(3):
            for mo in range(3):
                put(f"cc1_{tap}_{kc}_{mo}", w_t[kc * 128:(kc + 1) * 128, mo * 128:(mo + 1) * 128])
    wc2 = convc2_w[:, :, 0].T
    for kc in range(3):
        for mo in range(3):
            put(f"cc2_{kc}_{mo}", wc2[kc * 128:(kc + 1) * 128, mo * 128:(mo + 1) * 128])
    for kc in range(2):
        put(f"wy_{kc}", wy[kc * 128:(kc + 1) * 128, :])
    for kc in range(3):
        put(f"wz_{kc}", wz[kc * 128:(kc + 1) * 128, :])
    return img.astype(BF16)


def _pack_biases(bl, gru_bi, gru_bh, conv1_b, conv2_b, convc1_b, convc2_b, by, bz_):
    img = np.zeros((128, bl.col), np.float32)

    def put(name, vec, nch):
        o = bl.off[name]
        for c in range(nch):
            img[:, o + c] = vec[c * 128:(c + 1) * 128]

    put("br", gru_bi[0:256] + gru_bh[0:256], 2)
    put("bz", gru_bi[256:512] + gru_bh[256:512], 2)
    put("big", gru_bi[512:768], 2)
    put("bhg", gru_bh[512:768], 2)
    put("c1b", conv1_b, 2)
    put("c2b", conv2_b, 2)
    put("cc1b", convc1_b, 3)
    put("cc2b", convc2_b, 3)
    img[0, bl.off["by"]] = by[0]
    img[0, bl.off["bz_"]] = bz_[0]
    return img


def _pack_fp8_weights(W_msg, gru_Wi, gru_Wh):
    """fp8 DoubleRow lhsT images: [k 128, block, kc 2, m 128] where
    element [k, b, kc, m] = W[kc*128 + k, bcol(b)*128 + m]."""
    wf8 = np.zeros((128, NT * 2, 2, 128), np.float32)
    for t in range(NT):
        for mo in range(2):
            for kc in range(2):
                wf8[:, t * 2 + mo, kc, :] = W_msg[t][kc * 128:(kc + 1) * 128,
                                                     mo * 128:(mo + 1) * 128]
    wi8 = np.zeros((128, 6, 2, 128), np.float32)
    wh8 = np.zeros((128, 6, 2, 128), np.float32)
    for mi in range(6):
        for kc in range(2):
            wi8[:, mi, kc, :] = gru_Wi[kc * 128:(kc + 1) * 128, mi * 128:(mi + 1) * 128]
            wh8[:, mi, kc, :] = gru_Wh[kc * 128:(kc + 1) * 128, mi * 128:(mi + 1) * 128]
    return wf8.astype(FP8), wi8.astype(FP8), wh8.astype(FP8)


# ---------------------------------------------------------------------------
# edge preprocessing: bucket by (core, dst-window, etype), chunk by 128
# ---------------------------------------------------------------------------
def _preprocess_edges(src, dst, etype):
    core = dst // NPC
    w = (dst % NPC) // WIN
    t = etype
    dloc = (dst % WIN).astype(np.int64)
    key = (core * NWIN + w) * NT + t
    order = np.argsort(key, kind="stable")
    cnt = np.bincount(key, minlength=NCORES * NWIN * NT).reshape(NCORES, NWIN, NT)
    CB = np.maximum(1, -(-cnt // 128)).max(axis=0)      # [NWIN, NT]
    boff, SCBW, TOTCH, calls, ncoltot = _chunk_plan(CB)
    starts = np.zeros(NCORES * NWIN * NT + 1, np.int64)
    starts[1:] = np.cumsum(cnt.reshape(-1))
    ar128 = np.arange(128)

    S_img = np.zeros((NCORES, 128, TOTCH, 128), np.float32)
    L_img = np.zeros((NCORES, TOTCH * 128), np.int16)   # slot src ids
    for c in range(NCORES):
        for wdx in range(NWIN):
            for tt in range(NT):
                s0 = starts[(c * NWIN + wdx) * NT + tt]
                s1 = starts[(c * NWIN + wdx) * NT + tt + 1]
                seg = order[s0:s1]
                nch = int(CB[wdx][tt])
                pad = nch * 128 - len(seg)
                li = np.concatenate([src[seg].astype(np.int64),
                                     np.zeros(pad, np.int64)]).astype(np.int16)
                dl = np.concatenate([dloc[seg], np.full(pad, -1, np.int64)])
                b0 = boff[wdx][tt]
                L_img[c][b0 * 128:(b0 + nch) * 128] = li
                dlb = dl.reshape(nch, 128)
                for j in range(nch):
                    valid = dlb[j] >= 0
                    S_img[c][ar128[valid], b0 + j, dlb[j][valid]] = 1.0

    idx16 = np.zeros((NCORES, 128, ncoltot), np.int16)
    for c in range(NCORES):
        for g in range(NGRP):
            for (c0, c1, coloff) in calls[g]:
                flat = L_img[c][c0 * 128:c1 * 128]
                ncol = len(flat) // 16
                wrapped = flat.reshape(ncol, 16).T
                idx16[c][:, coloff:coloff + ncol] = np.tile(wrapped, (8, 1))
    return idx16, S_img.astype(FP8), CB


# ---------------------------------------------------------------------------
# device program
# ---------------------------------------------------------------------------
def build_program(CB, wl, bl, num_devices=NCORES, sim_mode=False, nsteps=NSTEPS):
    boff, SCBW, TOTCH, calls, ncoltot = _chunk_plan(CB)
    NCHG = [SCBW[g * GW] + SCBW[g * GW + 1] for g in range(NGRP)]
    NCHMAX = max(NCHG)
    nc = bacc.Bacc("TRN2", target_bir_lowering=False, debug=False,
                   num_devices=num_devices, num_swdge_queues=NQ)
    h0T_d = nc.dram_tensor("h0T", [OUT, NPC], bf, kind="ExternalInput")
    h0r8_d = nc.dram_tensor("h0r8", [NPC, OUT], fp8, kind="ExternalInput")
    h08_d = nc.dram_tensor("h08", [128, 2, NPC], fp8, kind="ExternalInput")
    xT_d = nc.dram_tensor("xT", [IN_DIM, NPC], bf, kind="ExternalInput")
    Wc_d = nc.dram_tensor("Wc", [128, wl.col], bf, kind="ExternalInput")
    Wf8_d = nc.dram_tensor("Wf8", [128, NT * 2, 2, 128], fp8, kind="ExternalInput")
    Wi8_d = nc.dram_tensor("Wi8", [128, 6, 2, 128], fp8, kind="ExternalInput")
    Wh8_d = nc.dram_tensor("Wh8", [128, 6, 2, 128], fp8, kind="ExternalInput")
    Bimg_d = nc.dram_tensor("Bimg", [128, bl.col], f32, kind="ExternalInput")
    Smat_d = nc.dram_tensor("Smat", [128, TOTCH, 128], fp8, kind="ExternalInput")
    idx_d = nc.dram_tensor("idx", [128, ncoltot], i16, kind="ExternalInput")
    mask_d = nc.dram_tensor("mask", [1, 1024], f32, kind="ExternalInput")
    ident_d = nc.dram_tensor("ident", [128, 128], bf, kind="ExternalInput")
    out_d = nc.dram_tensor("out", [GPC], f32, kind="ExternalOutput")

    with tile.TileContext(nc) as tc:
        with tc.tile_pool(name="persist", bufs=1) as pp, \
             tc.tile_pool(name="dram", bufs=1, space="DRAM") as dpool:
            hT0 = pp.tile([128, NPC], bf)
            hT1 = pp.tile([128, NPC], bf)
            h8 = pp.tile([128, 2, NPC], fp8)
            Wcs = pp.tile([128, wl.col], bf)
            Wf8 = pp.tile([128, NT * 2, 2, 128], fp8)
            Wi8 = pp.tile([128, 6, 2, 128], fp8)
            Wh8 = pp.tile([128, 6, 2, 128], fp8)
            Bsb = pp.tile([128, bl.col], f32)
            ident = pp.tile([128, 128], bf)
            nc.sync.dma_start(hT0[:], h0T_d.ap()[0:128, :])
            nc.sync.dma_start(hT1[:], h0T_d.ap()[128:256, :])
            nc.sync.dma_start(h8[:], h08_d.ap())
            nc.sync.dma_start(Wcs[:], Wc_d.ap())
            nc.sync.dma_start(Wf8[:], Wf8_d.ap())
            nc.sync.dma_start(Wi8[:], Wi8_d.ap())
            nc.sync.dma_start(Wh8[:], Wh8_d.ap())
            nc.sync.dma_start(Bsb[:], Bimg_d.ap())
            nc.sync.dma_start(ident[:], ident_d.ap())
            hT = [hT0, hT1]

            def W(name):
                o = wl.off[name]
                return Wcs[:, o:o + (1 if name[0] == 'w' else 128)]

            def Bias(name, c=0):
                o = bl.off[name] + c
                return Bsb[:, o:o + 1]

            def Bias1(name):
                o = bl.off[name]
                return Bsb[0:1, o:o + 1]

            in_cc0 = dpool.tile([NPC, OUT], fp8, name="in_cc0")
            nc.sync.dma_start(in_cc0[:], h0r8_d.ap())
            in_cc = [dpool.tile([NPC, OUT], fp8, name=f"in_cc{s}")
                     for s in range(1, nsteps)]
            out_cc = [dpool.tile([NN, OUT], fp8, addr_space="Shared",
                                 name=f"out_cc{s}") for s in range(nsteps)]

            # ---------------- GGNN loop ----------------
            with tc.tile_pool(name="sload", bufs=1) as slp, \
                 tc.tile_pool(name="gat", bufs=4) as gap, \
                 tc.tile_pool(name="stage", bufs=2) as stp, \
                 tc.tile_pool(name="gtmp", bufs=2) as gtp, \
                 tc.tile_pool(name="pA", bufs=1, space="PSUM") as pA, \
                 tc.tile_pool(name="pT", bufs=1, space="PSUM") as pT, \
                 tc.tile_pool(name="pG", bufs=1, space="PSUM") as pG, \
                 tc.tile_pool(name="pX", bufs=2, space="PSUM") as pX:
                S_sb = slp.tile([128, TOTCH, 128], fp8)
                idx_sb = slp.tile([128, ncoltot], i16)
                nc.sync.dma_start(S_sb[:], Smat_d.ap())
                nc.sync.dma_start(idx_sb[:], idx_d.ap())

                qi = 0
                for s in range(nsteps):
                    with nc.named_scope(f"s{s}_ag"):
                        ag_in = in_cc0[:] if s == 0 else in_cc[s - 1][:]
                        if sim_mode:
                            nc.sync.dma_start(out_cc[s][0:NPC, :], ag_in)
                        else:
                            nc.gpsimd.collective_compute(
                                "AllGather", ALU.bypass,
                                replica_groups=[list(range(num_devices))],
                                ins=[ag_in.opt()],
                                outs=[out_cc[s].opt()],
                            )
                    ctx_edge = nc.named_scope(f"s{s}_edge")
                    ctx_edge.__enter__()
                    for grp in range(NGRP):
                        gts = []       # (tile, c0) per call
                        for ci, (c0, c1, coloff) in enumerate(calls[grp]):
                            gtile = gap.tile([128, CALLCH, OUT], fp8,
                                             tag=f"gt{ci}", name=f"gt{ci}")
                            nrow = (c1 - c0) * 128
                            nc.gpsimd.dma_gather(
                                gtile[:, 0:c1 - c0, :],
                                out_cc[s][:],
                                idx_sb[:, coloff:coloff + nrow // 16],
                                nrow, nrow, OUT, queue_num=qi)
                            qi = (qi + 1) % NQ
                            gts.append((gtile, c0, c1))

                        def call_for(chunk):
                            for (gtile, c0, c1) in gts:
                                if c0 <= chunk < c1:
                                    return gtile, c0
                            raise AssertionError(chunk)

                        agg8 = [gtp.tile([128, 2, 256], fp8, tag=f"agg8_{t}",
                                         name=f"agg8_{t}") for t in range(NT)]
                        aT8 = gtp.tile([128, 2, 256], fp8, tag="aT8")
                        for wi in range(GW):
                            w = grp * GW + wi
                            ps_agg = pA.tile([128, NT, 2, 128], f32, space="PSUM",
                                             tag="agg")
                            for t in range(NT):
                                cb = int(CB[w][t])
                                o = boff[w][t]
                                gtile, gc0 = call_for(o)
                                npair = cb // 2
                                odd = cb % 2
                                for fc in range(2):
                                    for j in range(npair):
                                        nc.tensor.matmul(
                                            ps_agg[:, t, fc, :],
                                            lhsT=gtile[:, o - gc0 + 2 * j:o - gc0 + 2 * j + 2,
                                                       fc * 128:(fc + 1) * 128],
                                            rhs=S_sb[:, o + 2 * j:o + 2 * j + 2, :],
                                            start=(j == 0), stop=(j == npair - 1 and not odd),
                                            perf_mode=DR)
                                    if odd:
                                        nc.tensor.matmul(
                                            ps_agg[:, t, fc, :],
                                            lhsT=gtile[:, o - gc0 + cb - 1,
                                                       fc * 128:(fc + 1) * 128],
                                            rhs=S_sb[:, o + cb - 1, :],
                                            start=(npair == 0), stop=True)
                            for t in range(NT):
                                if t % 2 == 0:
                                    nc.vector.tensor_copy(
                                        agg8[t][:, :, wi * 128:(wi + 1) * 128],
                                        ps_agg[:, t, :, :])
                                else:
                                    nc.scalar.activation(
                                        agg8[t][:, :, wi * 128:(wi + 1) * 128],
                                        ps_agg[:, t, :, :], AF.Copy)
                        ps_aT = pT.tile([128, 2, 256], f32, space="PSUM", tag="aT")
                        for mo in range(2):
                            for t in range(NT):
                                nc.tensor.matmul(
                                    ps_aT[:, mo, :],
                                    lhsT=Wf8[:, t * 2 + mo, :, :],
                                    rhs=agg8[t][:],
                                    start=(t == 0), stop=(t == NT - 1),
                                    perf_mode=DR)
                        nc.vector.tensor_copy(aT8[:], ps_aT[:])
                        # ---- GRU on block nodes [grp*256, +256) ----
                        nb = grp * GW * WIN
                        h8blk = h8[:, :, nb:nb + 256]
                        for fc in range(2):
                            ps = pG.tile([128, 1024], f32, space="PSUM", tag="g")
                            for gi, col in ((0, 0), (1, 256)):
                                mi = gi * 2 + fc
                                nc.tensor.matmul(ps[:, col:col + 256],
                                                 lhsT=Wi8[:, mi, :, :], rhs=aT8[:],
                                                 start=True, stop=False, perf_mode=DR)
                                nc.tensor.matmul(ps[:, col:col + 256],
                                                 lhsT=Wh8[:, mi, :, :], rhs=h8blk,
                                                 start=False, stop=True, perf_mode=DR)
                            mi = 4 + fc
                            nc.tensor.matmul(ps[:, 512:768],
                                             lhsT=Wi8[:, mi, :, :], rhs=aT8[:],
                                             start=True, stop=True, perf_mode=DR)
                            nc.tensor.matmul(ps[:, 768:1024],
                                             lhsT=Wh8[:, mi, :, :], rhs=h8blk,
                                             start=True, stop=True, perf_mode=DR)
                            # stage gates to SBUF fast so the PSUM frees and
                            # the next block's matmuls can proceed
                            gsb = gtp.tile([128, 1024], bf, tag=f"gsb{fc}",
                                           name=f"gsb{fc}")
                            nc.vector.tensor_copy(gsb[:, 0:512], ps[:, 0:512])
                            nc.scalar.activation(gsb[:, 512:1024],
                                                 ps[:, 512:1024], AF.Copy)
                            hslice = hT[fc][:, nb:nb + 256]
                            r = gtp.tile([128, 256], bf, tag="r")
                            z = gtp.tile([128, 256], bf, tag="z")
                            t1 = gtp.tile([128, 256], f32, tag="t1")
                            g = gtp.tile([128, 256], f32, tag="g2")
                            d = gtp.tile([128, 256], f32, tag="d")
                            nc.scalar.activation(r[:], gsb[:, 0:256], AF.Sigmoid,
                                                 bias=Bias("br", fc))
                            nc.scalar.activation(z[:], gsb[:, 256:512], AF.Sigmoid,
                                                 bias=Bias("bz", fc))
                            nc.vector.scalar_tensor_tensor(
                                t1[:], gsb[:, 768:1024], Bias("bhg", fc), r[:],
                                op0=ALU.add, op1=ALU.mult)
                            nc.vector.tensor_add(t1[:], t1[:], gsb[:, 512:768])
                            nc.scalar.activation(g[:], t1[:], AF.Tanh,
                                                 bias=Bias("big", fc))
                            nc.vector.tensor_sub(d[:], hslice, g[:])
                            nc.vector.tensor_mul(d[:], z[:], d[:])
                            nc.vector.tensor_add(hslice, g[:], d[:])
                            # refresh fp8 interleaved state
                            nc.scalar.activation(h8[:, fc, nb:nb + 256], hslice,
                                                 AF.Copy)
                    # rows for the next step's AllGather: batched
                    # epilogue so PE transposes never head-block the grp loop
                    if s < nsteps - 1:
                        for sub in range(NPC // 128):
                            n0 = sub * 128
                            rows8 = stp.tile([128, 2, 128], fp8, tag=f"rows8_{sub % 2}",
                                             name="rows8")
                            for kc in range(2):
                                psT = pX.tile([128, 128], bf, space="PSUM",
                                              tag="T")
                                nc.tensor.matmul(psT[:],
                                                 lhsT=hT[kc][:, n0:n0 + 128],
                                                 rhs=ident[:],
                                                 start=True, stop=True,
                                                 is_transpose=True)
                                if kc == 0:
                                    nc.vector.tensor_copy(rows8[:, kc, :], psT[:])
                                else:
                                    nc.scalar.activation(rows8[:, kc, :], psT[:],
                                                         AF.Copy)
                            nc.sync.dma_start(in_cc[s][n0:n0 + 128, :], rows8[:])
                    ctx_edge.__exit__(None, None, None)

            # ---------------- readout (batched over 16 graphs) ----------------
            with tc.tile_pool(name="rsb", bufs=1) as rsb, \
                 tc.tile_pool(name="rtmp", bufs=1) as rtp, \
                 tc.tile_pool(name="pR", bufs=2, space="PSUM") as pR, \
                 tc.tile_pool(name="pV", bufs=1, space="PSUM") as pV:
                ctx_ro = nc.named_scope("readout")
                ctx_ro.__enter__()
                res_sb = pp.tile([1, GPC], f32)
                xTb = rsb.tile([128, NPC], bf)
                mask_sb = rsb.tile([1, 1024], f32)
                nc.sync.dma_start(xTb[:], xT_d.ap())
                nc.sync.dma_start(mask_sb[:], mask_d.ap())

                W1 = 4094          # conv1 output cols computed (of 4096)
                P1 = 2046          # pool1 output cols
                P2 = 1023          # pool2 output cols

                # --- Y path ---
                y1 = []
                for mo in range(2):
                    yt = rsb.tile([128, W1], bf, tag=f"w1_{mo}")
                    for ch0 in range(0, W1, 512):
                        cw = min(512, W1 - ch0)
                        psc = pR.tile([128, 512], f32, space="PSUM", tag="conv")
                        k = 0
                        for tap in range(3):
                            for kc in range(2):
                                nc.tensor.matmul(
                                    psc[:, 0:cw],
                                    lhsT=W(f"c1_{tap}_{kc}_{mo}"),
                                    rhs=hT[kc][:, ch0 + tap:ch0 + tap + cw],
                                    start=(k == 0), stop=(k == 5))
                                k += 1
                        nc.scalar.activation(yt[:, ch0:ch0 + cw], psc[:, 0:cw],
                                             AF.Relu, bias=Bias("c1b", mo))
                    y1.append(yt)

                def pool3(src, width, outw, tag):
                    # out[q] = max(src[2q], src[2q+1], src[2q+2]); q < outw
                    pout = rsb.tile([128, outw], bf, tag=tag)
                    tmp = rtp.tile([128, outw], bf, tag=tag + "t")
                    ab = src[:, 0:2 * outw].rearrange("p (n t) -> p n t", t=2)
                    nc.vector.tensor_max(tmp[:], ab[:, :, 0], ab[:, :, 1])
                    a2 = src[:, 2:2 * outw + 2].rearrange("p (n t) -> p n t", t=2)[:, :, 0]
                    nc.vector.tensor_max(pout[:], tmp[:], a2)
                    return pout

                def pool2(src, width, outw, tag):
                    pout = rsb.tile([128, outw], bf, tag=tag)
                    ab = src[:, 0:2 * outw].rearrange("p (n t) -> p n t", t=2)
                    nc.vector.tensor_max(pout[:], ab[:, :, 0], ab[:, :, 1])
                    return pout

                y2 = [pool3(y1[mo], W1, P1, f"p1_{mo}") for mo in range(2)]
                y3 = []
                for mo in range(2):
                    yt = rsb.tile([128, P1], bf, tag=f"c2_{mo}")
                    for ch0 in range(0, P1, 512):
                        cw = min(512, P1 - ch0)
                        psc = pR.tile([128, 512], f32, space="PSUM", tag="conv")
                        for kc in range(2):
                            nc.tensor.matmul(psc[:, 0:cw],
                                             lhsT=W(f"c2_{kc}_{mo}"),
                                             rhs=y2[kc][:, ch0:ch0 + cw],
                                             start=(kc == 0), stop=(kc == 1))
                        nc.scalar.activation(yt[:, ch0:ch0 + cw], psc[:, 0:cw],
                                             AF.Relu, bias=Bias("c2b", mo))
                    y3.append(yt)
                y4 = [pool2(y3[mo], P1, P2, f"p2_{mo}") for mo in range(2)]
                psy = pV.tile([1, 1024], f32, space="PSUM", tag="psy")
                for ch0 in range(0, P2, 512):
                    cw = min(512, P2 - ch0)
                    for kc in range(2):
                        nc.tensor.matmul(psy[:, ch0:ch0 + cw],
                                         lhsT=W(f"wy_{kc}"),
                                         rhs=y4[kc][:, ch0:ch0 + cw],
                                         start=(kc == 0), stop=(kc == 1))
                # --- Z path ---
                cch = [hT[0], hT[1], xTb]
                z1 = []
                for mo in range(3):
                    zt = rsb.tile([128, W1], bf, tag=f"w1_{mo}")
                    for ch0 in range(0, W1, 512):
                        cw = min(512, W1 - ch0)
                        psc = pR.tile([128, 512], f32, space="PSUM", tag="conv")
                        k = 0
                        for tap in range(3):
                            for kc in range(3):
                                nc.tensor.matmul(
                                    psc[:, 0:cw],
                                    lhsT=W(f"cc1_{tap}_{kc}_{mo}"),
                                    rhs=cch[kc][:, ch0 + tap:ch0 + tap + cw],
                                    start=(k == 0), stop=(k == 8))
                                k += 1
                        nc.scalar.activation(zt[:, ch0:ch0 + cw], psc[:, 0:cw],
                                             AF.Relu, bias=Bias("cc1b", mo))
                    z1.append(zt)
                z2 = [pool3(z1[mo], W1, P1, f"p1_{mo}") for mo in range(3)]
                z3 = []
                for mo in range(3):
                    zt = rsb.tile([128, P1], bf, tag=f"c2_{mo}")
                    for ch0 in range(0, P1, 512):
                        cw = min(512, P1 - ch0)
                        psc = pR.tile([128, 512], f32, space="PSUM", tag="conv")
                        for kc in range(3):
                            nc.tensor.matmul(psc[:, 0:cw],
                                             lhsT=W(f"cc2_{kc}_{mo}"),
                                             rhs=z2[kc][:, ch0:ch0 + cw],
                                             start=(kc == 0), stop=(kc == 2))
                        nc.scalar.activation(zt[:, ch0:ch0 + cw], psc[:, 0:cw],
                                             AF.Relu, bias=Bias("cc2b", mo))
                    z3.append(zt)
                z4 = [pool2(z3[mo], P1, P2, f"p2_{mo}") for mo in range(3)]
                psz = pV.tile([1, 1024], f32, space="PSUM", tag="psz")
                for ch0 in range(0, P2, 512):
                    cw = min(512, P2 - ch0)
                    for kc in range(3):
                        nc.tensor.matmul(psz[:, ch0:ch0 + cw],
                                         lhsT=W(f"wz_{kc}"),
                                         rhs=z4[kc][:, ch0:ch0 + cw],
                                         start=(kc == 0), stop=(kc == 2))
                # --- combine ---
                ty = rsb.tile([1, 1024], f32, tag="ty")
                tz = rsb.tile([1, 1024], f32, tag="tz")
                pr = rsb.tile([1, 1024], f32, tag="pr")
                sm = rsb.tile([1, GPC], f32, tag="sm")
                nc.vector.memset(ty[:], 0.0)
                nc.vector.memset(tz[:], 0.0)
                nc.vector.tensor_scalar_add(ty[:, 0:P2], psy[:, 0:P2], Bias1("by"))
                nc.vector.tensor_scalar_add(tz[:, 0:P2], psz[:, 0:P2], Bias1("bz_"))
                nc.vector.tensor_mul(pr[:], ty[:], tz[:])
                nc.vector.tensor_mul(pr[:], pr[:], mask_sb[:])
                nc.vector.tensor_reduce(
                    sm[:], pr[:].rearrange("p (g c) -> p g c", c=64),
                    axis=mybir.AxisListType.X, op=ALU.add)
                nc.scalar.activation(res_sb[0:1, :], sm[:],
                                     AF.Sigmoid, scale=1.0 / 63.0)
                nc.sync.dma_start(out_d.ap(), res_sb[0:1, :])
                ctx_ro.__exit__(None, None, None)
    nc.finalize()
    return nc


# ---------------------------------------------------------------------------
# host entry
# ---------------------------------------------------------------------------
def _prepare(inputs):
    features = np.asarray(inputs["features"], np.float32)
    src = np.asarray(inputs["src"]).astype(np.int64)
    dst = np.asarray(inputs["dst"]).astype(np.int64)
    etype = np.asarray(inputs["etype"]).astype(np.int64)
    wl = _make_wlayout()
    bl = _make_blayout()
    Wc = _pack_weights(
        wl,
        np.asarray(inputs["conv1_w"], np.float32),
        np.asarray(inputs["conv2_w"], np.float32),
        np.asarray(inputs["convc1_w"], np.float32),
        np.asarray(inputs["convc2_w"], np.float32),
        np.asarray(inputs["wy"], np.float32),
        np.asarray(inputs["wz"], np.float32),
    )
    Bimg = _pack_biases(
        bl,
        np.asarray(inputs["gru_bi"], np.float32),
        np.asarray(inputs["gru_bh"], np.float32),
        np.asarray(inputs["conv1_b"], np.float32),
        np.asarray(inputs["conv2_b"], np.float32),
        np.asarray(inputs["convc1_b"], np.float32),
        np.asarray(inputs["convc2_b"], np.float32),
        np.asarray(inputs["by"], np.float32),
        np.asarray(inputs["bz"], np.float32),
    )
    Wf8, Wi8, Wh8 = _pack_fp8_weights(
        np.asarray(inputs["W_msg"], np.float32),
        np.asarray(inputs["gru_Wi"], np.float32),
        np.asarray(inputs["gru_Wh"], np.float32),
    )
    b_msg = np.asarray(inputs["b_msg"], np.float32)
    assert np.abs(b_msg).max() == 0.0, "nonzero b_msg not supported"

    idx16, S_img, CB = _preprocess_edges(src, dst, etype)

    mask = np.zeros((1, 1024), np.float32)
    for gg in range(GPC):
        mask[0, gg * 64:gg * 64 + 63] = 1.0
    ident = np.eye(128).astype(BF16)

    in_maps = []
    for c in range(NCORES):
        feats = features[c * NPC:(c + 1) * NPC]  # [4096, 128]
        h0 = np.zeros((NPC, OUT), np.float32)
        h0[:, :IN_DIM] = feats
        h08 = np.zeros((128, 2, NPC), np.float32)
        h08[:, 0, :] = h0[:, 0:128].T
        h08[:, 1, :] = h0[:, 128:256].T
        im = {
            "h0T": h0.T.astype(BF16),
            "h0r8": h0.astype(FP8),
            "h08": h08.astype(FP8),
            "xT": feats.T.astype(BF16),
            "Wc": Wc,
            "Wf8": Wf8,
            "Wi8": Wi8,
            "Wh8": Wh8,
            "Bimg": Bimg,
            "Smat": S_img[c],
            "idx": idx16[c],
            "mask": mask,
            "ident": ident,
        }
        in_maps.append(im)
    return wl, bl, CB, in_maps


def kernel(**inputs):
    wl, bl, CB, in_maps = _prepare(inputs)
    nc = build_program(CB, wl, bl)
    res = run_bass_kernel_spmd(nc, in_maps, core_ids=list(range(NCORES)))
    out = np.concatenate([res.results[c]["out"] for c in range(NCORES)])
    return out.astype(np.float32)


# revision 28
# speedup vs baseline: 1.0321x; 1.0321x over previous
"""Devign GGNN model on 8 Trainium2 NeuronCores (Bass/Tile) — v2.

Strategy (data-parallel over dst-node shards, aggregate-first message
passing, fp8 message path):
  - 8 cores, core c owns nodes [c*4096, (c+1)*4096) = graphs [c*16, +16).
  - Node state hT kept transposed on-chip ([256 feat, 4096 nodes] bf16)
    plus an fp8 kc-interleaved copy h8 [128, 2, 4096] for DoubleRow GRU.
  - Per GGNN step:
      1. AllGather the raw fp8 node states: in_cc [4096, 256] fp8 ->
         out_cc [32768, 256] fp8 (8x less wire traffic than gathering the
         per-etype transformed table in bf16).
      2. Edges (sorted by (dst-window, etype), chunked into 128-edge
         slots) are processed per 2-window group: indirect-DMA row gather
         of h[src] (fp8 256B rows, 4 SWDGE queues round-robin), then a
         one-hot scatter matmul agg_t += chunk^T @ S per (window, etype)
         using fp8 DoubleRow (2 chunks per matmul), S resident in SBUF.
      3. Transform after aggregation: aT[:, w] = sum_t W_msg[t]^T @ agg_t
         (fp8 DoubleRow) — 4x fewer transform FLOPs than transforming
         every node by every etype.
      4. GRU: all six gate matmuls as fp8 DoubleRow (contraction 256 in
         one instruction); gate pointwise + state update in bf16/f32.
      5. New h rows for the next AllGather produced by PE transpose +
         fp8 cast per 128-node chunk.
  - Readout (conv1d/maxpool stacks + gated sum) evaluated batched over
    all 16 graphs at full width with tap-shifted matmuls; invalid
    cross-graph columns masked before the final per-graph reduction.

All index/one-hot/weight-layout preprocessing is done on the host at
kernel() time and baked into the compiled program + per-core inputs.
"""
import sys
import numpy as np

for _p in ("/opt/trn_rl_repo",):
    if _p not in sys.path:
        sys.path.insert(0, _p)

import ml_dtypes

import concourse.bass as bass
import concourse.mybir as mybir
import concourse.tile as tile
from concourse import bacc
from concourse.bass_utils import run_bass_kernel_spmd

BF16 = ml_dtypes.bfloat16
FP8 = ml_dtypes.float8_e4m3
F32 = np.float32

NCORES = 8
NN = 32768          # total nodes
IN_DIM = 128
OUT = 256
NT = 4              # edge types
NSTEPS = 8
NGRAPH = 128
NPC = NN // NCORES  # nodes per core = 4096
WIN = 128           # dst window size
NWIN = NPC // WIN   # 32 windows per core
GW = 2              # windows per group / GRU block of 256 nodes
NGRP = NWIN // GW   # 16 groups
GPC = NGRAPH // NCORES  # graphs per core = 16
LG = 256            # nodes per graph
CONCAT = IN_DIM + OUT  # 384
CALLCH = 8          # chunks per dma_gather call (1024 rows)
NQ = 4              # SWDGE queues

bf = mybir.dt.bfloat16
f32 = mybir.dt.float32
fp8 = mybir.dt.float8e4
i16 = mybir.dt.int16
AF = mybir.ActivationFunctionType
ALU = mybir.AluOpType
DR = mybir.MatmulPerfMode.DoubleRow


# ---------------------------------------------------------------------------
# chunk layout shared between host packer and device builder
# ---------------------------------------------------------------------------
def _chunk_plan(CB):
    """CB: [NWIN][NT] chunk counts. Returns (boff[w][t] global chunk offset,
    SCBW[w], TOTCH, calls: list over grp of list of (c0, c1, coloff))."""
    CB = [[int(x) for x in row] for row in CB]
    boff = []
    tot = 0
    SCBW = []
    for w in range(NWIN):
        row = []
        for t in range(NT):
            row.append(tot)
            tot += CB[w][t]
        boff.append(row)
        SCBW.append(sum(CB[w]))
    TOTCH = tot
    # Gather calls aligned to (window, etype-pair) buckets so each call's
    # data is an independent tile the scatter can start on immediately.
    calls = []
    coloff = 0
    for g in range(NGRP):
        grp_calls = []
        for wi in range(GW):
            w = g * GW + wi
            for tp in range(2):
                c0 = boff[w][tp * 2]
                c1 = c0 + CB[w][tp * 2] + CB[w][tp * 2 + 1]
                if c1 - c0 > CALLCH:  # split oversized pair per etype
                    cm = c0 + CB[w][tp * 2]
                    grp_calls.append((c0, cm, coloff))
                    coloff += (cm - c0) * 128 // 16
                    grp_calls.append((cm, c1, coloff))
                    coloff += (c1 - cm) * 128 // 16
                else:
                    grp_calls.append((c0, c1, coloff))
                    coloff += (c1 - c0) * 128 // 16
        calls.append(grp_calls)
    return boff, SCBW, TOTCH, calls, coloff


# ---------------------------------------------------------------------------
# weight/bias image layout (bf16 conv image)
# ---------------------------------------------------------------------------
class WLayout:
    def __init__(self):
        self.col = 0
        self.off = {}

    def alloc(self, name, width):
        self.off[name] = self.col
        self.col += width
        return self.off[name]


def _make_wlayout():
    wl = WLayout()
    for tap in range(3):
        for kc in range(2):
            for mo in range(2):
                wl.alloc(f"c1_{tap}_{kc}_{mo}", 128)
    for kc in range(2):
        for mo in range(2):
            wl.alloc(f"c2_{kc}_{mo}", 128)
    for tap in range(3):
        for kc in range(3):
            for mo in range(3):
                wl.alloc(f"cc1_{tap}_{kc}_{mo}", 128)
    for kc in range(3):
        for mo in range(3):
            wl.alloc(f"cc2_{kc}_{mo}", 128)
    for kc in range(2):
        wl.alloc(f"wy_{kc}", 1)
    for kc in range(3):
        wl.alloc(f"wz_{kc}", 1)
    return wl


def _make_blayout():
    bl = WLayout()
    for name in ("br", "bz", "big", "bhg"):
        bl.alloc(name, 2)       # [256] as two [128] cols
    bl.alloc("c1b", 2)
    bl.alloc("c2b", 2)
    bl.alloc("cc1b", 3)
    bl.alloc("cc2b", 3)
    bl.alloc("by", 1)
    bl.alloc("bz_", 1)
    return bl


def _pack_weights(wl, conv1_w, conv2_w, convc1_w, convc2_w, wy, wz):
    img = np.zeros((128, wl.col), np.float32)

    def put(name, block):
        o = wl.off[name]
        img[:, o:o + block.shape[1]] = block

    for tap in range(3):
        w_t = conv1_w[:, :, tap].T  # [i, o]
        for kc in range(2):
            for mo in range(2):
                put(f"c1_{tap}_{kc}_{mo}", w_t[kc * 128:(kc + 1) * 128, mo * 128:(mo + 1) * 128])
    w2 = conv2_w[:, :, 0].T
    for kc in range(2):
        for mo in range(2):
            put(f"c2_{kc}_{mo}", w2[kc * 128:(kc + 1) * 128, mo * 128:(mo + 1) * 128])
    for tap in range(3):
        w_t = convc1_w[:, :, tap].T
        for kc in range(3):
            for mo in range(3):
                put(f"cc1_{tap}_{kc}_{mo}", w_t[kc * 128:(kc + 1) * 128, mo * 128:(mo + 1) * 128])
    wc2 = convc2_w[:, :, 0].T
    for kc in range(3):
        for mo in range(3):
            put(f"cc2_{kc}_{mo}", wc2[kc * 128:(kc + 1) * 128, mo * 128:(mo + 1) * 128])
    for kc in range(2):
        put(f"wy_{kc}", wy[kc * 128:(kc + 1) * 128, :])
    for kc in range(3):
        put(f"wz_{kc}", wz[kc * 128:(kc + 1) * 128, :])
    return img.astype(BF16)


def _pack_biases(bl, gru_bi, gru_bh, conv1_b, conv2_b, convc1_b, convc2_b, by, bz_):
    img = np.zeros((128, bl.col), np.float32)

    def put(name, vec, nch):
        o = bl.off[name]
        for c in range(nch):
            img[:, o + c] = vec[c * 128:(c + 1) * 128]

    put("br", gru_bi[0:256] + gru_bh[0:256], 2)
    put("bz", gru_bi[256:512] + gru_bh[256:512], 2)
    put("big", gru_bi[512:768], 2)
    put("bhg", gru_bh[512:768], 2)
    put("c1b", conv1_b, 2)
    put("c2b", conv2_b, 2)
    put("cc1b", convc1_b, 3)
    put("cc2b", convc2_b, 3)
    img[0, bl.off["by"]] = by[0]
    img[0, bl.off["bz_"]] = bz_[0]
    return img


def _pack_fp8_weights(W_msg, gru_Wi, gru_Wh):
    """fp8 DoubleRow lhsT images: [k 128, block, kc 2, m 128] where
    element [k, b, kc, m] = W[kc*128 + k, bcol(b)*128 + m]."""
    wf8 = np.zeros((128, NT * 2, 2, 128), np.float32)
    for t in range(NT):
        for mo in range(2):
            for kc in range(2):
                wf8[:, t * 2 + mo, kc, :] = W_msg[t][kc * 128:(kc + 1) * 128,
                                                     mo * 128:(mo + 1) * 128]
    wi8 = np.zeros((128, 6, 2, 128), np.float32)
    wh8 = np.zeros((128, 6, 2, 128), np.float32)
    for mi in range(6):
        for kc in range(2):
            wi8[:, mi, kc, :] = gru_Wi[kc * 128:(kc + 1) * 128, mi * 128:(mi + 1) * 128]
            wh8[:, mi, kc, :] = gru_Wh[kc * 128:(kc + 1) * 128, mi * 128:(mi + 1) * 128]
    return wf8.astype(FP8), wi8.astype(FP8), wh8.astype(FP8)


# ---------------------------------------------------------------------------
# edge preprocessing: bucket by (core, dst-window, etype), chunk by 128
# ---------------------------------------------------------------------------
def _preprocess_edges(src, dst, etype):
    core = dst // NPC
    w = (dst % NPC) // WIN
    t = etype
    dloc = (dst % WIN).astype(np.int64)
    key = (core * NWIN + w) * NT + t
    order = np.argsort(key, kind="stable")
    cnt = np.bincount(key, minlength=NCORES * NWIN * NT).reshape(NCORES, NWIN, NT)
    CB = np.maximum(1, -(-cnt // 128)).max(axis=0)      # [NWIN, NT]
    boff, SCBW, TOTCH, calls, ncoltot = _chunk_plan(CB)
    starts = np.zeros(NCORES * NWIN * NT + 1, np.int64)
    starts[1:] = np.cumsum(cnt.reshape(-1))
    ar128 = np.arange(128)

    S_img = np.zeros((NCORES, 128, TOTCH, 128), np.float32)
    L_img = np.zeros((NCORES, TOTCH * 128), np.int16)   # slot src ids
    for c in range(NCORES):
        for wdx in range(NWIN):
            for tt in range(NT):
                s0 = starts[(c * NWIN + wdx) * NT + tt]
                s1 = starts[(c * NWIN + wdx) * NT + tt + 1]
                seg = order[s0:s1]
                nch = int(CB[wdx][tt])
                pad = nch * 128 - len(seg)
                li = np.concatenate([src[seg].astype(np.int64),
                                     np.zeros(pad, np.int64)]).astype(np.int16)
                dl = np.concatenate([dloc[seg], np.full(pad, -1, np.int64)])
                b0 = boff[wdx][tt]
                L_img[c][b0 * 128:(b0 + nch) * 128] = li
                dlb = dl.reshape(nch, 128)
                for j in range(nch):
                    valid = dlb[j] >= 0
                    S_img[c][ar128[valid], b0 + j, dlb[j][valid]] = 1.0

    idx16 = np.zeros((NCORES, 128, ncoltot), np.int16)
    for c in range(NCORES):
        for g in range(NGRP):
            for (c0, c1, coloff) in calls[g]:
                flat = L_img[c][c0 * 128:c1 * 128]
                ncol = len(flat) // 16
                wrapped = flat.reshape(ncol, 16).T
                idx16[c][:, coloff:coloff + ncol] = np.tile(wrapped, (8, 1))
    return idx16, S_img.astype(FP8), CB


# ---------------------------------------------------------------------------
# device program
# ---------------------------------------------------------------------------
def build_program(CB, wl, bl, num_devices=NCORES, sim_mode=False, nsteps=NSTEPS):
    boff, SCBW, TOTCH, calls, ncoltot = _chunk_plan(CB)
    NCHG = [SCBW[g * GW] + SCBW[g * GW + 1] for g in range(NGRP)]
    NCHMAX = max(NCHG)
    nc = bacc.Bacc("TRN2", target_bir_lowering=False, debug=False,
                   num_devices=num_devices, num_swdge_queues=NQ)
    h0T_d = nc.dram_tensor("h0T", [OUT, NPC], bf, kind="ExternalInput")
    h0r8_d = nc.dram_tensor("h0r8", [NPC, OUT], fp8, kind="ExternalInput")
    h08_d = nc.dram_tensor("h08", [128, 2, NPC], fp8, kind="ExternalInput")
    xT_d = nc.dram_tensor("xT", [IN_DIM, NPC], bf, kind="ExternalInput")
    Wc_d = nc.dram_tensor("Wc", [128, wl.col], bf, kind="ExternalInput")
    Wf8_d = nc.dram_tensor("Wf8", [128, NT * 2, 2, 128], fp8, kind="ExternalInput")
    Wi8_d = nc.dram_tensor("Wi8", [128, 6, 2, 128], fp8, kind="ExternalInput")
    Wh8_d = nc.dram_tensor("Wh8", [128, 6, 2, 128], fp8, kind="ExternalInput")
    Bimg_d = nc.dram_tensor("Bimg", [128, bl.col], f32, kind="ExternalInput")
    Smat_d = nc.dram_tensor("Smat", [128, TOTCH, 128], fp8, kind="ExternalInput")
    idx_d = nc.dram_tensor("idx", [128, ncoltot], i16, kind="ExternalInput")
    mask_d = nc.dram_tensor("mask", [1, 1024], f32, kind="ExternalInput")
    ident_d = nc.dram_tensor("ident", [128, 128], bf, kind="ExternalInput")
    out_d = nc.dram_tensor("out", [GPC], f32, kind="ExternalOutput")

    with tile.TileContext(nc) as tc:
        with tc.tile_pool(name="persist", bufs=1) as pp, \
             tc.tile_pool(name="dram", bufs=1, space="DRAM") as dpool:
            hT0 = pp.tile([128, NPC], bf)
            hT1 = pp.tile([128, NPC], bf)
            h8 = pp.tile([128, 2, NPC], fp8)
            Wcs = pp.tile([128, wl.col], bf)
            Wf8 = pp.tile([128, NT * 2, 2, 128], fp8)
            Wi8 = pp.tile([128, 6, 2, 128], fp8)
            Wh8 = pp.tile([128, 6, 2, 128], fp8)
            Bsb = pp.tile([128, bl.col], f32)
            ident = pp.tile([128, 128], bf)
            nc.sync.dma_start(hT0[:], h0T_d.ap()[0:128, :])
            nc.sync.dma_start(hT1[:], h0T_d.ap()[128:256, :])
            nc.sync.dma_start(h8[:], h08_d.ap())
            nc.sync.dma_start(Wcs[:], Wc_d.ap())
            nc.sync.dma_start(Wf8[:], Wf8_d.ap())
            nc.sync.dma_start(Wi8[:], Wi8_d.ap())
            nc.sync.dma_start(Wh8[:], Wh8_d.ap())
            nc.sync.dma_start(Bsb[:], Bimg_d.ap())
            nc.sync.dma_start(ident[:], ident_d.ap())
            hT = [hT0, hT1]

            def W(name):
                o = wl.off[name]
                return Wcs[:, o:o + (1 if name[0] == 'w' else 128)]

            def Bias(name, c=0):
                o = bl.off[name] + c
                return Bsb[:, o:o + 1]

            def Bias1(name):
                o = bl.off[name]
                return Bsb[0:1, o:o + 1]

            in_cc0 = dpool.tile([NPC, OUT], fp8, name="in_cc0")
            nc.sync.dma_start(in_cc0[:], h0r8_d.ap())
            in_cc = [dpool.tile([NPC, OUT], fp8, name=f"in_cc{s}")
                     for s in range(1, nsteps)]
            out_cc = [dpool.tile([NN, OUT], fp8, addr_space="Shared",
                                 name=f"out_cc{s}") for s in range(nsteps)]

            # ---------------- GGNN loop ----------------
            with tc.tile_pool(name="sload", bufs=1) as slp, \
                 tc.tile_pool(name="gat", bufs=2) as gap, \
                 tc.tile_pool(name="stage", bufs=2) as stp, \
                 tc.tile_pool(name="gtmp", bufs=2) as gtp, \
                 tc.tile_pool(name="pA", bufs=1, space="PSUM") as pA, \
                 tc.tile_pool(name="pT", bufs=1, space="PSUM") as pT, \
                 tc.tile_pool(name="pG", bufs=1, space="PSUM") as pG, \
                 tc.tile_pool(name="pX", bufs=1, space="PSUM") as pX:
                S_sb = slp.tile([128, TOTCH, 128], fp8)
                idx_sb = slp.tile([128, ncoltot], i16)
                nc.sync.dma_start(S_sb[:], Smat_d.ap())
                nc.sync.dma_start(idx_sb[:], idx_d.ap())

                qi = 0
                for s in range(nsteps):
                    with nc.named_scope(f"s{s}_ag"):
                        ag_in = in_cc0[:] if s == 0 else in_cc[s - 1][:]
                        if sim_mode:
                            nc.sync.dma_start(out_cc[s][0:NPC, :], ag_in)
                        else:
                            nc.gpsimd.collective_compute(
                                "AllGather", ALU.bypass,
                                replica_groups=[list(range(num_devices))],
                                ins=[ag_in.opt()],
                                outs=[out_cc[s].opt()],
                            )
                    ctx_edge = nc.named_scope(f"s{s}_edge")
                    ctx_edge.__enter__()
                    for grp in range(NGRP):
                        gts = []       # (tile, c0) per call
                        for ci, (c0, c1, coloff) in enumerate(calls[grp]):
                            gtile = gap.tile([128, CALLCH, OUT], fp8,
                                             tag=f"gt{ci}", name=f"gt{ci}")
                            nrow = (c1 - c0) * 128
                            nc.gpsimd.dma_gather(
                                gtile[:, 0:c1 - c0, :],
                                out_cc[s][:],
                                idx_sb[:, coloff:coloff + nrow // 16],
                                nrow, nrow, OUT, queue_num=qi)
                            qi = (qi + 1) % NQ
                            gts.append((gtile, c0, c1))

                        def call_for(chunk):
                            for (gtile, c0, c1) in gts:
                                if c0 <= chunk < c1:
                                    return gtile, c0
                            raise AssertionError(chunk)

                        agg8 = [gtp.tile([128, 2, 256], fp8, tag=f"agg8_{t}",
                                         name=f"agg8_{t}") for t in range(NT)]
                        aT8 = gtp.tile([128, 2, 256], fp8, tag="aT8")
                        for wi in range(GW):
                            w = grp * GW + wi
                            ps_agg = pA.tile([128, NT, 2, 128], f32, space="PSUM",
                                             tag="agg")
                            for t in range(NT):
                                cb = int(CB[w][t])
                                o = boff[w][t]
                                gtile, gc0 = call_for(o)
                                npair = cb // 2
                                odd = cb % 2
                                for fc in range(2):
                                    for j in range(npair):
                                        nc.tensor.matmul(
                                            ps_agg[:, t, fc, :],
                                            lhsT=gtile[:, o - gc0 + 2 * j:o - gc0 + 2 * j + 2,
                                                       fc * 128:(fc + 1) * 128],
                                            rhs=S_sb[:, o + 2 * j:o + 2 * j + 2, :],
                                            start=(j == 0), stop=(j == npair - 1 and not odd),
                                            perf_mode=DR)
                                    if odd:
                                        nc.tensor.matmul(
                                            ps_agg[:, t, fc, :],
                                            lhsT=gtile[:, o - gc0 + cb - 1,
                                                       fc * 128:(fc + 1) * 128],
                                            rhs=S_sb[:, o + cb - 1, :],
                                            start=(npair == 0), stop=True)
                            for t in range(NT):
                                if t % 2 == 0:
                                    nc.vector.tensor_copy(
                                        agg8[t][:, :, wi * 128:(wi + 1) * 128],
                                        ps_agg[:, t, :, :])
                                else:
                                    nc.scalar.activation(
                                        agg8[t][:, :, wi * 128:(wi + 1) * 128],
                                        ps_agg[:, t, :, :], AF.Copy)
                        ps_aT = pT.tile([128, 2, 256], f32, space="PSUM", tag="aT")
                        for mo in range(2):
                            for t in range(NT):
                                nc.tensor.matmul(
                                    ps_aT[:, mo, :],
                                    lhsT=Wf8[:, t * 2 + mo, :, :],
                                    rhs=agg8[t][:],
                                    start=(t == 0), stop=(t == NT - 1),
                                    perf_mode=DR)
                        nc.vector.tensor_copy(aT8[:], ps_aT[:])
                        # ---- GRU on block nodes [grp*256, +256) ----
                        nb = grp * GW * WIN
                        h8blk = h8[:, :, nb:nb + 256]
                        for fc in range(2):
                            ps = pG.tile([128, 1024], f32, space="PSUM", tag=f"g{fc}")
                            for gi, col in ((0, 0), (1, 256)):
                                mi = gi * 2 + fc
                                nc.tensor.matmul(ps[:, col:col + 256],
                                                 lhsT=Wi8[:, mi, :, :], rhs=aT8[:],
                                                 start=True, stop=False, perf_mode=DR)
                                nc.tensor.matmul(ps[:, col:col + 256],
                                                 lhsT=Wh8[:, mi, :, :], rhs=h8blk,
                                                 start=False, stop=True, perf_mode=DR)
                            mi = 4 + fc
                            nc.tensor.matmul(ps[:, 512:768],
                                             lhsT=Wi8[:, mi, :, :], rhs=aT8[:],
                                             start=True, stop=True, perf_mode=DR)
                            nc.tensor.matmul(ps[:, 768:1024],
                                             lhsT=Wh8[:, mi, :, :], rhs=h8blk,
                                             start=True, stop=True, perf_mode=DR)
                            hslice = hT[fc][:, nb:nb + 256]
                            r = gtp.tile([128, 256], bf, tag="r")
                            z = gtp.tile([128, 256], bf, tag="z")
                            t1 = gtp.tile([128, 256], f32, tag="t1")
                            g = gtp.tile([128, 256], f32, tag="g2")
                            d = gtp.tile([128, 256], f32, tag="d")
                            nc.scalar.activation(r[:], ps[:, 0:256], AF.Sigmoid,
                                                 bias=Bias("br", fc))
                            nc.scalar.activation(z[:], ps[:, 256:512], AF.Sigmoid,
                                                 bias=Bias("bz", fc))
                            nc.vector.scalar_tensor_tensor(
                                t1[:], ps[:, 768:1024], Bias("bhg", fc), r[:],
                                op0=ALU.add, op1=ALU.mult)
                            nc.vector.tensor_add(t1[:], t1[:], ps[:, 512:768])
                            nc.scalar.activation(g[:], t1[:], AF.Tanh,
                                                 bias=Bias("big", fc))
                            nc.vector.tensor_sub(d[:], hslice, g[:])
                            nc.vector.tensor_mul(d[:], z[:], d[:])
                            nc.vector.tensor_add(hslice, g[:], d[:])
                            # refresh fp8 interleaved state
                            nc.scalar.activation(h8[:, fc, nb:nb + 256], hslice,
                                                 AF.Copy)
                        # rows for the next step's AllGather
                        if s < nsteps - 1:
                            for sub in range(GW):
                                n0 = nb + sub * 128
                                rows8 = stp.tile([128, 2, 128], fp8, tag="rows8",
                                                 name="rows8")
                                for kc in range(2):
                                    psT = pX.tile([128, 128], bf, space="PSUM",
                                                  tag="T")
                                    nc.tensor.matmul(psT[:],
                                                     lhsT=hT[kc][:, n0:n0 + 128],
                                                     rhs=ident[:],
                                                     start=True, stop=True,
                                                     is_transpose=True)
                                    if kc == 0:
                                        nc.vector.tensor_copy(rows8[:, kc, :], psT[:])
                                    else:
                                        nc.scalar.activation(rows8[:, kc, :], psT[:],
                                                             AF.Copy)
                                nc.sync.dma_start(in_cc[s][n0:n0 + 128, :], rows8[:])
                    ctx_edge.__exit__(None, None, None)

            # ---------------- readout (batched over 16 graphs) ----------------
            with tc.tile_pool(name="rsb", bufs=1) as rsb, \
                 tc.tile_pool(name="rtmp", bufs=1) as rtp, \
                 tc.tile_pool(name="pR", bufs=2, space="PSUM") as pR, \
                 tc.tile_pool(name="pV", bufs=1, space="PSUM") as pV:
                ctx_ro = nc.named_scope("readout")
                ctx_ro.__enter__()
                res_sb = pp.tile([1, GPC], f32)
                xTb = rsb.tile([128, NPC], bf)
                mask_sb = rsb.tile([1, 1024], f32)
                nc.sync.dma_start(xTb[:], xT_d.ap())
                nc.sync.dma_start(mask_sb[:], mask_d.ap())

                W1 = 4094          # conv1 output cols computed (of 4096)
                P1 = 2046          # pool1 output cols
                P2 = 1023          # pool2 output cols

                # --- Y path ---
                y1 = []
                for mo in range(2):
                    yt = rsb.tile([128, W1], bf, tag=f"w1_{mo}")
                    for ch0 in range(0, W1, 512):
                        cw = min(512, W1 - ch0)
                        psc = pR.tile([128, 512], f32, space="PSUM", tag="conv")
                        k = 0
                        for tap in range(3):
                            for kc in range(2):
                                nc.tensor.matmul(
                                    psc[:, 0:cw],
                                    lhsT=W(f"c1_{tap}_{kc}_{mo}"),
                                    rhs=hT[kc][:, ch0 + tap:ch0 + tap + cw],
                                    start=(k == 0), stop=(k == 5))
                                k += 1
                        nc.scalar.activation(yt[:, ch0:ch0 + cw], psc[:, 0:cw],
                                             AF.Relu, bias=Bias("c1b", mo))
                    y1.append(yt)

                def pool3(src, width, outw, tag):
                    # out[q] = max(src[2q], src[2q+1], src[2q+2]); q < outw
                    pout = rsb.tile([128, outw], bf, tag=tag)
                    tmp = rtp.tile([128, outw], bf, tag=tag + "t")
                    ab = src[:, 0:2 * outw].rearrange("p (n t) -> p n t", t=2)
                    nc.vector.tensor_max(tmp[:], ab[:, :, 0], ab[:, :, 1])
                    a2 = src[:, 2:2 * outw + 2].rearrange("p (n t) -> p n t", t=2)[:, :, 0]
                    nc.vector.tensor_max(pout[:], tmp[:], a2)
                    return pout

                def pool2(src, width, outw, tag):
                    pout = rsb.tile([128, outw], bf, tag=tag)
                    ab = src[:, 0:2 * outw].rearrange("p (n t) -> p n t", t=2)
                    nc.vector.tensor_max(pout[:], ab[:, :, 0], ab[:, :, 1])
                    return pout

                y2 = [pool3(y1[mo], W1, P1, f"p1_{mo}") for mo in range(2)]
                y3 = []
                for mo in range(2):
                    yt = rsb.tile([128, P1], bf, tag=f"c2_{mo}")
                    for ch0 in range(0, P1, 512):
                        cw = min(512, P1 - ch0)
                        psc = pR.tile([128, 512], f32, space="PSUM", tag="conv")
                        for kc in range(2):
                            nc.tensor.matmul(psc[:, 0:cw],
                                             lhsT=W(f"c2_{kc}_{mo}"),
                                             rhs=y2[kc][:, ch0:ch0 + cw],
                                             start=(kc == 0), stop=(kc == 1))
                        nc.scalar.activation(yt[:, ch0:ch0 + cw], psc[:, 0:cw],
                                             AF.Relu, bias=Bias("c2b", mo))
                    y3.append(yt)
                y4 = [pool2(y3[mo], P1, P2, f"p2_{mo}") for mo in range(2)]
                psy = pV.tile([1, 1024], f32, space="PSUM", tag="psy")
                for ch0 in range(0, P2, 512):
                    cw = min(512, P2 - ch0)
                    for kc in range(2):
                        nc.tensor.matmul(psy[:, ch0:ch0 + cw],
                                         lhsT=W(f"wy_{kc}"),
                                         rhs=y4[kc][:, ch0:ch0 + cw],
                                         start=(kc == 0), stop=(kc == 1))
                # --- Z path ---
                cch = [hT[0], hT[1], xTb]
                z1 = []
                for mo in range(3):
                    zt = rsb.tile([128, W1], bf, tag=f"w1_{mo}")
                    for ch0 in range(0, W1, 512):
                        cw = min(512, W1 - ch0)
                        psc = pR.tile([128, 512], f32, space="PSUM", tag="conv")
                        k = 0
                        for tap in range(3):
                            for kc in range(3):
                                nc.tensor.matmul(
                                    psc[:, 0:cw],
                                    lhsT=W(f"cc1_{tap}_{kc}_{mo}"),
                                    rhs=cch[kc][:, ch0 + tap:ch0 + tap + cw],
                                    start=(k == 0), stop=(k == 8))
                                k += 1
                        nc.scalar.activation(zt[:, ch0:ch0 + cw], psc[:, 0:cw],
                                             AF.Relu, bias=Bias("cc1b", mo))
                    z1.append(zt)
                z2 = [pool3(z1[mo], W1, P1, f"p1_{mo}") for mo in range(3)]
                z3 = []
                for mo in range(3):
                    zt = rsb.tile([128, P1], bf, tag=f"c2_{mo}")
                    for ch0 in range(0, P1, 512):
                        cw = min(512, P1 - ch0)
                        psc = pR.tile([128, 512], f32, space="PSUM", tag="conv")
                        for kc in range(3):
                            nc.tensor.matmul(psc[:, 0:cw],
                                             lhsT=W(f"cc2_{kc}_{mo}"),
                                             rhs=z2[kc][:, ch0:ch0 + cw],
                                             start=(kc == 0), stop=(kc == 2))
                        nc.scalar.activation(zt[:, ch0:ch0 + cw], psc[:, 0:cw],
                                             AF.Relu, bias=Bias("cc2b", mo))
                    z3.append(zt)
                z4 = [pool2(z3[mo], P1, P2, f"p2_{mo}") for mo in range(3)]
                psz = pV.tile([1, 1024], f32, space="PSUM", tag="psz")
                for ch0 in range(0, P2, 512):
                    cw = min(512, P2 - ch0)
                    for kc in range(3):
                        nc.tensor.matmul(psz[:, ch0:ch0 + cw],
                                         lhsT=W(f"wz_{kc}"),
                                         rhs=z4[kc][:, ch0:ch0 + cw],
                                         start=(kc == 0), stop=(kc == 2))
                # --- combine ---
                ty = rsb.tile([1, 1024], f32, tag="ty")
                tz = rsb.tile([1, 1024], f32, tag="tz")
                pr = rsb.tile([1, 1024], f32, tag="pr")
                sm = rsb.tile([1, GPC], f32, tag="sm")
                nc.vector.memset(ty[:], 0.0)
                nc.vector.memset(tz[:], 0.0)
                nc.vector.tensor_scalar_add(ty[:, 0:P2], psy[:, 0:P2], Bias1("by"))
                nc.vector.tensor_scalar_add(tz[:, 0:P2], psz[:, 0:P2], Bias1("bz_"))
                nc.vector.tensor_mul(pr[:], ty[:], tz[:])
                nc.vector.tensor_mul(pr[:], pr[:], mask_sb[:])
                nc.vector.tensor_reduce(
                    sm[:], pr[:].rearrange("p (g c) -> p g c", c=64),
                    axis=mybir.AxisListType.X, op=ALU.add)
                nc.scalar.activation(res_sb[0:1, :], sm[:],
                                     AF.Sigmoid, scale=1.0 / 63.0)
                nc.sync.dma_start(out_d.ap(), res_sb[0:1, :])
                ctx_ro.__exit__(None, None, None)
    nc.finalize()
    return nc


# ---------------------------------------------------------------------------
# host entry
# ---------------------------------------------------------------------------
def _prepare(inputs):
    features = np.asarray(inputs["features"], np.float32)
    src = np.asarray(inputs["src"]).astype(np.int64)
    dst = np.asarray(inputs["dst"]).astype(np.int64)
    etype = np.asarray(inputs["etype"]).astype(np.int64)
    wl = _make_wlayout()
    bl = _make_blayout()
    Wc = _pack_weights(
        wl,
        np.asarray(inputs["conv1_w"], np.float32),
        np.asarray(inputs["conv2_w"], np.float32),
        np.asarray(inputs["convc1_w"], np.float32),
        np.asarray(inputs["convc2_w"], np.float32),
        np.asarray(inputs["wy"], np.float32),
        np.asarray(inputs["wz"], np.float32),
    )
    Bimg = _pack_biases(
        bl,
        np.asarray(inputs["gru_bi"], np.float32),
        np.asarray(inputs["gru_bh"], np.float32),
        np.asarray(inputs["conv1_b"], np.float32),
        np.asarray(inputs["conv2_b"], np.float32),
        np.asarray(inputs["convc1_b"], np.float32),
        np.asarray(inputs["convc2_b"], np.float32),
        np.asarray(inputs["by"], np.float32),
        np.asarray(inputs["bz"], np.float32),
    )
    Wf8, Wi8, Wh8 = _pack_fp8_weights(
        np.asarray(inputs["W_msg"], np.float32),
        np.asarray(inputs["gru_Wi"], np.float32),
        np.asarray(inputs["gru_Wh"], np.float32),
    )
    b_msg = np.asarray(inputs["b_msg"], np.float32)
    assert np.abs(b_msg).max() == 0.0, "nonzero b_msg not supported"

    idx16, S_img, CB = _preprocess_edges(src, dst, etype)

    mask = np.zeros((1, 1024), np.float32)
    for gg in range(GPC):
        mask[0, gg * 64:gg * 64 + 63] = 1.0
    ident = np.eye(128).astype(BF16)

    in_maps = []
    for c in range(NCORES):
        feats = features[c * NPC:(c + 1) * NPC]  # [4096, 128]
        h0 = np.zeros((NPC, OUT), np.float32)
        h0[:, :IN_DIM] = feats
        h08 = np.zeros((128, 2, NPC), np.float32)
        h08[:, 0, :] = h0[:, 0:128].T
        h08[:, 1, :] = h0[:, 128:256].T
        im = {
            "h0T": h0.T.astype(BF16),
            "h0r8": h0.astype(FP8),
            "h08": h08.astype(FP8),
            "xT": feats.T.astype(BF16),
            "Wc": Wc,
            "Wf8": Wf8,
            "Wi8": Wi8,
            "Wh8": Wh8,
            "Bimg": Bimg,
            "Smat": S_img[c],
            "idx": idx16[c],
            "mask": mask,
            "ident": ident,
        }
        in_maps.append(im)
    return wl, bl, CB, in_maps


def kernel(**inputs):
    wl, bl, CB, in_maps = _prepare(inputs)
    nc = build_program(CB, wl, bl)
    res = run_bass_kernel_spmd(nc, in_maps, core_ids=list(range(NCORES)))
    out = np.concatenate([res.results[c]["out"] for c in range(NCORES)])
    return out.astype(np.float32)


# revision 29
# speedup vs baseline: 1.0939x; 1.0598x over previous
"""Devign GGNN model on 8 Trainium2 NeuronCores (Bass/Tile) — v2.

Strategy (data-parallel over dst-node shards, aggregate-first message
passing, fp8 message path):
  - 8 cores, core c owns nodes [c*4096, (c+1)*4096) = graphs [c*16, +16).
  - Node state hT kept transposed on-chip ([256 feat, 4096 nodes] bf16)
    plus an fp8 kc-interleaved copy h8 [128, 2, 4096] for DoubleRow GRU.
  - Per GGNN step:
      1. AllGather the raw fp8 node states: in_cc [4096, 256] fp8 ->
         out_cc [32768, 256] fp8 (8x less wire traffic than gathering the
         per-etype transformed table in bf16).
      2. Edges (sorted by (dst-window, etype), chunked into 128-edge
         slots) are processed per 2-window group: indirect-DMA row gather
         of h[src] (fp8 256B rows, 4 SWDGE queues round-robin), then a
         one-hot scatter matmul agg_t += chunk^T @ S per (window, etype)
         using fp8 DoubleRow (2 chunks per matmul), S resident in SBUF.
      3. Transform after aggregation: aT[:, w] = sum_t W_msg[t]^T @ agg_t
         (fp8 DoubleRow) — 4x fewer transform FLOPs than transforming
         every node by every etype.
      4. GRU: all six gate matmuls as fp8 DoubleRow (contraction 256 in
         one instruction); gate pointwise + state update in bf16/f32.
      5. New h rows for the next AllGather produced by PE transpose +
         fp8 cast per 128-node chunk.
  - Readout (conv1d/maxpool stacks + gated sum) evaluated batched over
    all 16 graphs at full width with tap-shifted matmuls; invalid
    cross-graph columns masked before the final per-graph reduction.

All index/one-hot/weight-layout preprocessing is done on the host at
kernel() time and baked into the compiled program + per-core inputs.
"""
import sys
import numpy as np

for _p in ("/opt/trn_rl_repo",):
    if _p not in sys.path:
        sys.path.insert(0, _p)

import ml_dtypes

import concourse.bass as bass
import concourse.mybir as mybir
import concourse.tile as tile
from concourse import bacc
from concourse.bass_utils import run_bass_kernel_spmd

BF16 = ml_dtypes.bfloat16
FP8 = ml_dtypes.float8_e4m3
F32 = np.float32

NCORES = 8
NN = 32768          # total nodes
IN_DIM = 128
OUT = 256
NT = 4              # edge types
NSTEPS = 8
NGRAPH = 128
NPC = NN // NCORES  # nodes per core = 4096
WIN = 128           # dst window size
NWIN = NPC // WIN   # 32 windows per core
GW = 2              # windows per group / GRU block of 256 nodes
NGRP = NWIN // GW   # 16 groups
GPC = NGRAPH // NCORES  # graphs per core = 16
LG = 256            # nodes per graph
CONCAT = IN_DIM + OUT  # 384
CALLCH = 8          # chunks per dma_gather call (1024 rows)
NQ = 4              # SWDGE queues

bf = mybir.dt.bfloat16
f32 = mybir.dt.float32
fp8 = mybir.dt.float8e4
i16 = mybir.dt.int16
AF = mybir.ActivationFunctionType
ALU = mybir.AluOpType
DR = mybir.MatmulPerfMode.DoubleRow


# ---------------------------------------------------------------------------
# chunk layout shared between host packer and device builder
# ---------------------------------------------------------------------------
def _chunk_plan(CB):
    """CB: [NWIN][NT] chunk counts. Returns (boff[w][t] global chunk offset,
    SCBW[w], TOTCH, calls: list over grp of list of (c0, c1, coloff))."""
    CB = [[int(x) for x in row] for row in CB]
    boff = []
    tot = 0
    SCBW = []
    for w in range(NWIN):
        row = []
        for t in range(NT):
            row.append(tot)
            tot += CB[w][t]
        boff.append(row)
        SCBW.append(sum(CB[w]))
    TOTCH = tot
    calls = []
    coloff = 0
    for g in range(NGRP):
        c0 = boff[g * GW][0]
        c1 = c0 + SCBW[g * GW] + SCBW[g * GW + 1]
        grp_calls = []
        c = c0
        while c < c1:
            ce = min(c + CALLCH, c1)
            grp_calls.append((c, ce, coloff))
            coloff += (ce - c) * 128 // 16
            c = ce
        calls.append(grp_calls)
    return boff, SCBW, TOTCH, calls, coloff


# ---------------------------------------------------------------------------
# weight/bias image layout (bf16 conv image)
# ---------------------------------------------------------------------------
class WLayout:
    def __init__(self):
        self.col = 0
        self.off = {}

    def alloc(self, name, width):
        self.off[name] = self.col
        self.col += width
        return self.off[name]


def _make_wlayout():
    wl = WLayout()
    for tap in range(3):
        for kc in range(2):
            for mo in range(2):
                wl.alloc(f"c1_{tap}_{kc}_{mo}", 128)
    for kc in range(2):
        for mo in range(2):
            wl.alloc(f"c2_{kc}_{mo}", 128)
    for tap in range(3):
        for kc in range(3):
            for mo in range(3):
                wl.alloc(f"cc1_{tap}_{kc}_{mo}", 128)
    for kc in range(3):
        for mo in range(3):
            wl.alloc(f"cc2_{kc}_{mo}", 128)
    for kc in range(2):
        wl.alloc(f"wy_{kc}", 1)
    for kc in range(3):
        wl.alloc(f"wz_{kc}", 1)
    return wl


def _make_blayout():
    bl = WLayout()
    for name in ("br", "bz", "big", "bhg"):
        bl.alloc(name, 2)       # [256] as two [128] cols
    bl.alloc("c1b", 2)
    bl.alloc("c2b", 2)
    bl.alloc("cc1b", 3)
    bl.alloc("cc2b", 3)
    bl.alloc("by", 1)
    bl.alloc("bz_", 1)
    return bl


def _pack_weights(wl, conv1_w, conv2_w, convc1_w, convc2_w, wy, wz):
    img = np.zeros((128, wl.col), np.float32)

    def put(name, block):
        o = wl.off[name]
        img[:, o:o + block.shape[1]] = block

    for tap in range(3):
        w_t = conv1_w[:, :, tap].T  # [i, o]
        for kc in range(2):
            for mo in range(2):
                put(f"c1_{tap}_{kc}_{mo}", w_t[kc * 128:(kc + 1) * 128, mo * 128:(mo + 1) * 128])
    w2 = conv2_w[:, :, 0].T
    for kc in range(2):
        for mo in range(2):
            put(f"c2_{kc}_{mo}", w2[kc * 128:(kc + 1) * 128, mo * 128:(mo + 1) * 128])
    for tap in range(3):
        w_t = convc1_w[:, :, tap].T
        for kc in range(3):
            for mo in range(3):
                put(f"cc1_{tap}_{kc}_{mo}", w_t[kc * 128:(kc + 1) * 128, mo * 128:(mo + 1) * 128])
    wc2 = convc2_w[:, :, 0].T
    for kc in range(3):
        for mo in range(3):
            put(f"cc2_{kc}_{mo}", wc2[kc * 128:(kc + 1) * 128, mo * 128:(mo + 1) * 128])
    for kc in range(2):
        put(f"wy_{kc}", wy[kc * 128:(kc + 1) * 128, :])
    for kc in range(3):
        put(f"wz_{kc}", wz[kc * 128:(kc + 1) * 128, :])
    return img.astype(BF16)


def _pack_biases(bl, gru_bi, gru_bh, conv1_b, conv2_b, convc1_b, convc2_b, by, bz_):
    img = np.zeros((128, bl.col), np.float32)

    def put(name, vec, nch):
        o = bl.off[name]
        for c in range(nch):
            img[:, o + c] = vec[c * 128:(c + 1) * 128]

    put("br", gru_bi[0:256] + gru_bh[0:256], 2)
    put("bz", gru_bi[256:512] + gru_bh[256:512], 2)
    put("big", gru_bi[512:768], 2)
    put("bhg", gru_bh[512:768], 2)
    put("c1b", conv1_b, 2)
    put("c2b", conv2_b, 2)
    put("cc1b", convc1_b, 3)
    put("cc2b", convc2_b, 3)
    img[0, bl.off["by"]] = by[0]
    img[0, bl.off["bz_"]] = bz_[0]
    return img


def _pack_fp8_weights(W_msg, gru_Wi, gru_Wh):
    """fp8 DoubleRow lhsT images: [k 128, block, kc 2, m 128] where
    element [k, b, kc, m] = W[kc*128 + k, bcol(b)*128 + m]."""
    wf8 = np.zeros((128, NT * 2, 2, 128), np.float32)
    for t in range(NT):
        for mo in range(2):
            for kc in range(2):
                wf8[:, t * 2 + mo, kc, :] = W_msg[t][kc * 128:(kc + 1) * 128,
                                                     mo * 128:(mo + 1) * 128]
    wi8 = np.zeros((128, 6, 2, 128), np.float32)
    wh8 = np.zeros((128, 6, 2, 128), np.float32)
    for mi in range(6):
        for kc in range(2):
            wi8[:, mi, kc, :] = gru_Wi[kc * 128:(kc + 1) * 128, mi * 128:(mi + 1) * 128]
            wh8[:, mi, kc, :] = gru_Wh[kc * 128:(kc + 1) * 128, mi * 128:(mi + 1) * 128]
    return wf8.astype(FP8), wi8.astype(FP8), wh8.astype(FP8)


# ---------------------------------------------------------------------------
# edge preprocessing: bucket by (core, dst-window, etype), chunk by 128
# ---------------------------------------------------------------------------
def _preprocess_edges(src, dst, etype):
    core = dst // NPC
    w = (dst % NPC) // WIN
    t = etype
    dloc = (dst % WIN).astype(np.int64)
    key = (core * NWIN + w) * NT + t
    order = np.argsort(key, kind="stable")
    cnt = np.bincount(key, minlength=NCORES * NWIN * NT).reshape(NCORES, NWIN, NT)
    CB = np.maximum(1, -(-cnt // 128)).max(axis=0)      # [NWIN, NT]
    boff, SCBW, TOTCH, calls, ncoltot = _chunk_plan(CB)
    starts = np.zeros(NCORES * NWIN * NT + 1, np.int64)
    starts[1:] = np.cumsum(cnt.reshape(-1))
    ar128 = np.arange(128)

    S_img = np.zeros((NCORES, 128, TOTCH, 128), np.float32)
    L_img = np.zeros((NCORES, TOTCH * 128), np.int16)   # slot src ids
    for c in range(NCORES):
        for wdx in range(NWIN):
            for tt in range(NT):
                s0 = starts[(c * NWIN + wdx) * NT + tt]
                s1 = starts[(c * NWIN + wdx) * NT + tt + 1]
                seg = order[s0:s1]
                nch = int(CB[wdx][tt])
                pad = nch * 128 - len(seg)
                li = np.concatenate([src[seg].astype(np.int64),
                                     np.zeros(pad, np.int64)]).astype(np.int16)
                dl = np.concatenate([dloc[seg], np.full(pad, -1, np.int64)])
                b0 = boff[wdx][tt]
                L_img[c][b0 * 128:(b0 + nch) * 128] = li
                dlb = dl.reshape(nch, 128)
                for j in range(nch):
                    valid = dlb[j] >= 0
                    S_img[c][ar128[valid], b0 + j, dlb[j][valid]] = 1.0

    idx16 = np.zeros((NCORES, 128, ncoltot), np.int16)
    for c in range(NCORES):
        for g in range(NGRP):
            for (c0, c1, coloff) in calls[g]:
                flat = L_img[c][c0 * 128:c1 * 128]
                ncol = len(flat) // 16
                wrapped = flat.reshape(ncol, 16).T
                idx16[c][:, coloff:coloff + ncol] = np.tile(wrapped, (8, 1))
    return idx16, S_img.astype(FP8), CB


# ---------------------------------------------------------------------------
# device program
# ---------------------------------------------------------------------------
def build_program(CB, wl, bl, num_devices=NCORES, sim_mode=False, nsteps=NSTEPS):
    boff, SCBW, TOTCH, calls, ncoltot = _chunk_plan(CB)
    NCHG = [SCBW[g * GW] + SCBW[g * GW + 1] for g in range(NGRP)]
    NCHMAX = max(NCHG)
    nc = bacc.Bacc("TRN2", target_bir_lowering=False, debug=False,
                   num_devices=num_devices, num_swdge_queues=NQ)
    h0T_d = nc.dram_tensor("h0T", [OUT, NPC], bf, kind="ExternalInput")
    h0r8_d = nc.dram_tensor("h0r8", [NPC, OUT], fp8, kind="ExternalInput")
    h08_d = nc.dram_tensor("h08", [128, 2, NPC], fp8, kind="ExternalInput")
    xT_d = nc.dram_tensor("xT", [IN_DIM, NPC], bf, kind="ExternalInput")
    Wc_d = nc.dram_tensor("Wc", [128, wl.col], bf, kind="ExternalInput")
    Wf8_d = nc.dram_tensor("Wf8", [128, NT * 2, 2, 128], fp8, kind="ExternalInput")
    Wi8_d = nc.dram_tensor("Wi8", [128, 6, 2, 128], fp8, kind="ExternalInput")
    Wh8_d = nc.dram_tensor("Wh8", [128, 6, 2, 128], fp8, kind="ExternalInput")
    Bimg_d = nc.dram_tensor("Bimg", [128, bl.col], f32, kind="ExternalInput")
    Smat_d = nc.dram_tensor("Smat", [128, TOTCH, 128], fp8, kind="ExternalInput")
    idx_d = nc.dram_tensor("idx", [128, ncoltot], i16, kind="ExternalInput")
    mask_d = nc.dram_tensor("mask", [1, 1024], f32, kind="ExternalInput")
    ident_d = nc.dram_tensor("ident", [128, 128], bf, kind="ExternalInput")
    out_d = nc.dram_tensor("out", [GPC], f32, kind="ExternalOutput")

    with tile.TileContext(nc) as tc:
        with tc.tile_pool(name="persist", bufs=1) as pp, \
             tc.tile_pool(name="dram", bufs=1, space="DRAM") as dpool:
            hT0 = pp.tile([128, NPC], bf)
            hT1 = pp.tile([128, NPC], bf)
            h8 = pp.tile([128, 2, NPC], fp8)
            Wcs = pp.tile([128, wl.col], bf)
            Wf8 = pp.tile([128, NT * 2, 2, 128], fp8)
            Wi8 = pp.tile([128, 6, 2, 128], fp8)
            Wh8 = pp.tile([128, 6, 2, 128], fp8)
            Bsb = pp.tile([128, bl.col], f32)
            ident = pp.tile([128, 128], bf)
            nc.sync.dma_start(hT0[:], h0T_d.ap()[0:128, :])
            nc.sync.dma_start(hT1[:], h0T_d.ap()[128:256, :])
            nc.sync.dma_start(h8[:], h08_d.ap())
            nc.sync.dma_start(Wcs[:], Wc_d.ap())
            nc.sync.dma_start(Wf8[:], Wf8_d.ap())
            nc.sync.dma_start(Wi8[:], Wi8_d.ap())
            nc.sync.dma_start(Wh8[:], Wh8_d.ap())
            nc.sync.dma_start(Bsb[:], Bimg_d.ap())
            nc.sync.dma_start(ident[:], ident_d.ap())
            hT = [hT0, hT1]

            def W(name):
                o = wl.off[name]
                return Wcs[:, o:o + (1 if name[0] == 'w' else 128)]

            def Bias(name, c=0):
                o = bl.off[name] + c
                return Bsb[:, o:o + 1]

            def Bias1(name):
                o = bl.off[name]
                return Bsb[0:1, o:o + 1]

            in_cc0 = dpool.tile([NPC, OUT], fp8, name="in_cc0")
            nc.sync.dma_start(in_cc0[:], h0r8_d.ap())
            in_cc = [dpool.tile([NPC, OUT], fp8, name=f"in_cc{s}")
                     for s in range(1, nsteps)]
            out_cc = [dpool.tile([NN, OUT], fp8, addr_space="Shared",
                                 name=f"out_cc{s}") for s in range(nsteps)]

            # ---------------- GGNN loop ----------------
            with tc.tile_pool(name="sload", bufs=1) as slp, \
                 tc.tile_pool(name="gat", bufs=2) as gap, \
                 tc.tile_pool(name="stage", bufs=2) as stp, \
                 tc.tile_pool(name="gtmp", bufs=2) as gtp, \
                 tc.tile_pool(name="pA", bufs=1, space="PSUM") as pA, \
                 tc.tile_pool(name="pT", bufs=1, space="PSUM") as pT, \
                 tc.tile_pool(name="pG", bufs=1, space="PSUM") as pG, \
                 tc.tile_pool(name="pX", bufs=1, space="PSUM") as pX:
                S_sb = slp.tile([128, TOTCH, 128], fp8)
                idx_sb = slp.tile([128, ncoltot], i16)
                nc.sync.dma_start(S_sb[:], Smat_d.ap())
                nc.sync.dma_start(idx_sb[:], idx_d.ap())

                qi = 0
                for s in range(nsteps):
                    with nc.named_scope(f"s{s}_ag"):
                        ag_in = in_cc0[:] if s == 0 else in_cc[s - 1][:]
                        if sim_mode:
                            nc.sync.dma_start(out_cc[s][0:NPC, :], ag_in)
                        else:
                            nc.gpsimd.collective_compute(
                                "AllGather", ALU.bypass,
                                replica_groups=[list(range(num_devices))],
                                ins=[ag_in.opt()],
                                outs=[out_cc[s].opt()],
                            )
                    ctx_edge = nc.named_scope(f"s{s}_edge")
                    ctx_edge.__enter__()
                    for grp in range(NGRP):
                        gbase = boff[grp * GW][0]
                        gt = gap.tile([128, NCHMAX, OUT], fp8, tag="gt")
                        for (c0, c1, coloff) in calls[grp]:
                            nrow = (c1 - c0) * 128
                            nc.gpsimd.dma_gather(
                                gt[:, c0 - gbase:c1 - gbase, :],
                                out_cc[s][:],
                                idx_sb[:, coloff:coloff + nrow // 16],
                                nrow, nrow, OUT, queue_num=qi)
                            qi = (qi + 1) % NQ
                        aT8 = gtp.tile([128, 2, 256], fp8, tag="aT8")
                        for wi in range(GW):
                            w = grp * GW + wi
                            ps_agg = pA.tile([128, NT, 2, 128], f32, space="PSUM",
                                             tag="agg")
                            for t in range(NT):
                                cb = int(CB[w][t])
                                o = boff[w][t]
                                npair = cb // 2
                                odd = cb % 2
                                for fc in range(2):
                                    for j in range(npair):
                                        nc.tensor.matmul(
                                            ps_agg[:, t, fc, :],
                                            lhsT=gt[:, o - gbase + 2 * j:o - gbase + 2 * j + 2,
                                                    fc * 128:(fc + 1) * 128],
                                            rhs=S_sb[:, o + 2 * j:o + 2 * j + 2, :],
                                            start=(j == 0), stop=(j == npair - 1 and not odd),
                                            perf_mode=DR)
                                    if odd:
                                        nc.tensor.matmul(
                                            ps_agg[:, t, fc, :],
                                            lhsT=gt[:, o - gbase + cb - 1,
                                                    fc * 128:(fc + 1) * 128],
                                            rhs=S_sb[:, o + cb - 1, :],
                                            start=(npair == 0), stop=True)
                            agg8 = [gtp.tile([128, 2, 128], fp8, tag=f"agg8_{t}",
                                             name=f"agg8_{t}")
                                    for t in range(NT)]
                            for t in range(NT):
                                for fc in range(2):
                                    if (t + fc) % 2 == 0:
                                        nc.vector.tensor_copy(agg8[t][:, fc, :],
                                                              ps_agg[:, t, fc, :])
                                    else:
                                        nc.scalar.activation(agg8[t][:, fc, :],
                                                             ps_agg[:, t, fc, :], AF.Copy)
                            ps_aT = pT.tile([128, 2, 128], f32, space="PSUM", tag="aT")
                            for mo in range(2):
                                for t in range(NT):
                                    nc.tensor.matmul(
                                        ps_aT[:, mo, :],
                                        lhsT=Wf8[:, t * 2 + mo, :, :],
                                        rhs=agg8[t][:],
                                        start=(t == 0), stop=(t == NT - 1),
                                        perf_mode=DR)
                            for mo in range(2):
                                if mo == 0:
                                    nc.vector.tensor_copy(
                                        aT8[:, mo, wi * 128:(wi + 1) * 128],
                                        ps_aT[:, mo, :])
                                else:
                                    nc.scalar.activation(
                                        aT8[:, mo, wi * 128:(wi + 1) * 128],
                                        ps_aT[:, mo, :], AF.Copy)
                        # ---- GRU on block nodes [grp*256, +256) ----
                        nb = grp * GW * WIN
                        h8blk = h8[:, :, nb:nb + 256]
                        for fc in range(2):
                            ps = pG.tile([128, 1024], f32, space="PSUM", tag=f"g{fc}")
                            for gi, col in ((0, 0), (1, 256)):
                                mi = gi * 2 + fc
                                nc.tensor.matmul(ps[:, col:col + 256],
                                                 lhsT=Wi8[:, mi, :, :], rhs=aT8[:],
                                                 start=True, stop=False, perf_mode=DR)
                                nc.tensor.matmul(ps[:, col:col + 256],
                                                 lhsT=Wh8[:, mi, :, :], rhs=h8blk,
                                                 start=False, stop=True, perf_mode=DR)
                            mi = 4 + fc
                            nc.tensor.matmul(ps[:, 512:768],
                                             lhsT=Wi8[:, mi, :, :], rhs=aT8[:],
                                             start=True, stop=True, perf_mode=DR)
                            nc.tensor.matmul(ps[:, 768:1024],
                                             lhsT=Wh8[:, mi, :, :], rhs=h8blk,
                                             start=True, stop=True, perf_mode=DR)
                            hslice = hT[fc][:, nb:nb + 256]
                            r = gtp.tile([128, 256], bf, tag="r")
                            z = gtp.tile([128, 256], bf, tag="z")
                            t1 = gtp.tile([128, 256], f32, tag="t1")
                            g = gtp.tile([128, 256], f32, tag="g2")
                            d = gtp.tile([128, 256], f32, tag="d")
                            nc.scalar.activation(r[:], ps[:, 0:256], AF.Sigmoid,
                                                 bias=Bias("br", fc))
                            nc.scalar.activation(z[:], ps[:, 256:512], AF.Sigmoid,
                                                 bias=Bias("bz", fc))
                            nc.vector.scalar_tensor_tensor(
                                t1[:], ps[:, 768:1024], Bias("bhg", fc), r[:],
                                op0=ALU.add, op1=ALU.mult)
                            nc.vector.tensor_add(t1[:], t1[:], ps[:, 512:768])
                            nc.scalar.activation(g[:], t1[:], AF.Tanh,
                                                 bias=Bias("big", fc))
                            nc.vector.tensor_sub(d[:], hslice, g[:])
                            nc.vector.tensor_mul(d[:], z[:], d[:])
                            nc.vector.tensor_add(hslice, g[:], d[:])
                            # refresh fp8 interleaved state
                            nc.scalar.activation(h8[:, fc, nb:nb + 256], hslice,
                                                 AF.Copy)
                        # rows for the next step's AllGather
                        if s < nsteps - 1:
                            for sub in range(GW):
                                n0 = nb + sub * 128
                                rows8 = stp.tile([128, 2, 128], fp8, tag="rows8",
                                                 name="rows8")
                                for kc in range(2):
                                    psT = pX.tile([128, 128], bf, space="PSUM",
                                                  tag="T")
                                    nc.tensor.matmul(psT[:],
                                                     lhsT=hT[kc][:, n0:n0 + 128],
                                                     rhs=ident[:],
                                                     start=True, stop=True,
                                                     is_transpose=True)
                                    if kc == 0:
                                        nc.vector.tensor_copy(rows8[:, kc, :], psT[:])
                                    else:
                                        nc.scalar.activation(rows8[:, kc, :], psT[:],
                                                             AF.Copy)
                                nc.sync.dma_start(in_cc[s][n0:n0 + 128, :], rows8[:])
                    ctx_edge.__exit__(None, None, None)

            # ---------------- readout (batched over 16 graphs) ----------------
            with tc.tile_pool(name="rsb", bufs=1) as rsb, \
                 tc.tile_pool(name="rtmp", bufs=1) as rtp, \
                 tc.tile_pool(name="pR", bufs=2, space="PSUM") as pR, \
                 tc.tile_pool(name="pV", bufs=1, space="PSUM") as pV:
                ctx_ro = nc.named_scope("readout")
                ctx_ro.__enter__()
                res_sb = pp.tile([1, GPC], f32)
                xTb = rsb.tile([128, NPC], bf)
                mask_sb = rsb.tile([1, 1024], f32)
                nc.sync.dma_start(xTb[:], xT_d.ap())
                nc.sync.dma_start(mask_sb[:], mask_d.ap())

                W1 = 4094          # conv1 output cols computed (of 4096)
                P1 = 2046          # pool1 output cols
                P2 = 1023          # pool2 output cols

                # --- Y path ---
                y1 = []
                for mo in range(2):
                    yt = rsb.tile([128, W1], bf, tag=f"w1_{mo}")
                    for ch0 in range(0, W1, 512):
                        cw = min(512, W1 - ch0)
                        psc = pR.tile([128, 512], f32, space="PSUM", tag="conv")
                        k = 0
                        for tap in range(3):
                            for kc in range(2):
                                nc.tensor.matmul(
                                    psc[:, 0:cw],
                                    lhsT=W(f"c1_{tap}_{kc}_{mo}"),
                                    rhs=hT[kc][:, ch0 + tap:ch0 + tap + cw],
                                    start=(k == 0), stop=(k == 5))
                                k += 1
                        nc.scalar.activation(yt[:, ch0:ch0 + cw], psc[:, 0:cw],
                                             AF.Relu, bias=Bias("c1b", mo))
                    y1.append(yt)

                def pool3(src, width, outw, tag):
                    # out[q] = max(src[2q], src[2q+1], src[2q+2]); q < outw
                    pout = rsb.tile([128, outw], bf, tag=tag)
                    tmp = rtp.tile([128, outw], bf, tag=tag + "t")
                    ab = src[:, 0:2 * outw].rearrange("p (n t) -> p n t", t=2)
                    nc.vector.tensor_max(tmp[:], ab[:, :, 0], ab[:, :, 1])
                    a2 = src[:, 2:2 * outw + 2].rearrange("p (n t) -> p n t", t=2)[:, :, 0]
                    nc.vector.tensor_max(pout[:], tmp[:], a2)
                    return pout

                def pool2(src, width, outw, tag):
                    pout = rsb.tile([128, outw], bf, tag=tag)
                    ab = src[:, 0:2 * outw].rearrange("p (n t) -> p n t", t=2)
                    nc.vector.tensor_max(pout[:], ab[:, :, 0], ab[:, :, 1])
                    return pout

                y2 = [pool3(y1[mo], W1, P1, f"p1_{mo}") for mo in range(2)]
                y3 = []
                for mo in range(2):
                    yt = rsb.tile([128, P1], bf, tag=f"c2_{mo}")
                    for ch0 in range(0, P1, 512):
                        cw = min(512, P1 - ch0)
                        psc = pR.tile([128, 512], f32, space="PSUM", tag="conv")
                        for kc in range(2):
                            nc.tensor.matmul(psc[:, 0:cw],
                                             lhsT=W(f"c2_{kc}_{mo}"),
                                             rhs=y2[kc][:, ch0:ch0 + cw],
                                             start=(kc == 0), stop=(kc == 1))
                        nc.scalar.activation(yt[:, ch0:ch0 + cw], psc[:, 0:cw],
                                             AF.Relu, bias=Bias("c2b", mo))
                    y3.append(yt)
                y4 = [pool2(y3[mo], P1, P2, f"p2_{mo}") for mo in range(2)]
                psy = pV.tile([1, 1024], f32, space="PSUM", tag="psy")
                for ch0 in range(0, P2, 512):
                    cw = min(512, P2 - ch0)
                    for kc in range(2):
                        nc.tensor.matmul(psy[:, ch0:ch0 + cw],
                                         lhsT=W(f"wy_{kc}"),
                                         rhs=y4[kc][:, ch0:ch0 + cw],
                                         start=(kc == 0), stop=(kc == 1))
                # --- Z path ---
                cch = [hT[0], hT[1], xTb]
                z1 = []
                for mo in range(3):
                    zt = rsb.tile([128, W1], bf, tag=f"w1_{mo}")
                    for ch0 in range(0, W1, 512):
                        cw = min(512, W1 - ch0)
                        psc = pR.tile([128, 512], f32, space="PSUM", tag="conv")
                        k = 0
                        for tap in range(3):
                            for kc in range(3):
                                nc.tensor.matmul(
                                    psc[:, 0:cw],
                                    lhsT=W(f"cc1_{tap}_{kc}_{mo}"),
                                    rhs=cch[kc][:, ch0 + tap:ch0 + tap + cw],
                                    start=(k == 0), stop=(k == 8))
                                k += 1
                        nc.scalar.activation(zt[:, ch0:ch0 + cw], psc[:, 0:cw],
                                             AF.Relu, bias=Bias("cc1b", mo))
                    z1.append(zt)
                z2 = [pool3(z1[mo], W1, P1, f"p1_{mo}") for mo in range(3)]
                z3 = []
                for mo in range(3):
                    zt = rsb.tile([128, P1], bf, tag=f"c2_{mo}")
                    for ch0 in range(0, P1, 512):
                        cw = min(512, P1 - ch0)
                        psc = pR.tile([128, 512], f32, space="PSUM", tag="conv")
                        for kc in range(3):
                            nc.tensor.matmul(psc[:, 0:cw],
                                             lhsT=W(f"cc2_{kc}_{mo}"),
                                             rhs=z2[kc][:, ch0:ch0 + cw],
                                             start=(kc == 0), stop=(kc == 2))
                        nc.scalar.activation(zt[:, ch0:ch0 + cw], psc[:, 0:cw],
                                             AF.Relu, bias=Bias("cc2b", mo))
                    z3.append(zt)
                z4 = [pool2(z3[mo], P1, P2, f"p2_{mo}") for mo in range(3)]
                psz = pV.tile([1, 1024], f32, space="PSUM", tag="psz")
                for ch0 in range(0, P2, 512):
                    cw = min(512, P2 - ch0)
                    for kc in range(3):
                        nc.tensor.matmul(psz[:, ch0:ch0 + cw],
                                         lhsT=W(f"wz_{kc}"),
                                         rhs=z4[kc][:, ch0:ch0 + cw],
                                         start=(kc == 0), stop=(kc == 2))
                # --- combine ---
                ty = rsb.tile([1, 1024], f32, tag="ty")
                tz = rsb.tile([1, 1024], f32, tag="tz")
                pr = rsb.tile([1, 1024], f32, tag="pr")
                sm = rsb.tile([1, GPC], f32, tag="sm")
                nc.vector.memset(ty[:], 0.0)
                nc.vector.memset(tz[:], 0.0)
                nc.vector.tensor_scalar_add(ty[:, 0:P2], psy[:, 0:P2], Bias1("by"))
                nc.vector.tensor_scalar_add(tz[:, 0:P2], psz[:, 0:P2], Bias1("bz_"))
                nc.vector.tensor_mul(pr[:], ty[:], tz[:])
                nc.vector.tensor_mul(pr[:], pr[:], mask_sb[:])
                nc.vector.tensor_reduce(
                    sm[:], pr[:].rearrange("p (g c) -> p g c", c=64),
                    axis=mybir.AxisListType.X, op=ALU.add)
                nc.scalar.activation(res_sb[0:1, :], sm[:],
                                     AF.Sigmoid, scale=1.0 / 63.0)
                nc.sync.dma_start(out_d.ap(), res_sb[0:1, :])
                ctx_ro.__exit__(None, None, None)
    nc.finalize()
    return nc


# ---------------------------------------------------------------------------
# host entry
# ---------------------------------------------------------------------------
def _prepare(inputs):
    features = np.asarray(inputs["features"], np.float32)
    src = np.asarray(inputs["src"]).astype(np.int64)
    dst = np.asarray(inputs["dst"]).astype(np.int64)
    etype = np.asarray(inputs["etype"]).astype(np.int64)
    wl = _make_wlayout()
    bl = _make_blayout()
    Wc = _pack_weights(
        wl,
        np.asarray(inputs["conv1_w"], np.float32),
        np.asarray(inputs["conv2_w"], np.float32),
        np.asarray(inputs["convc1_w"], np.float32),
        np.asarray(inputs["convc2_w"], np.float32),
        np.asarray(inputs["wy"], np.float32),
        np.asarray(inputs["wz"], np.float32),
    )
    Bimg = _pack_biases(
        bl,
        np.asarray(inputs["gru_bi"], np.float32),
        np.asarray(inputs["gru_bh"], np.float32),
        np.asarray(inputs["conv1_b"], np.float32),
        np.asarray(inputs["conv2_b"], np.float32),
        np.asarray(inputs["convc1_b"], np.float32),
        np.asarray(inputs["convc2_b"], np.float32),
        np.asarray(inputs["by"], np.float32),
        np.asarray(inputs["bz"], np.float32),
    )
    Wf8, Wi8, Wh8 = _pack_fp8_weights(
        np.asarray(inputs["W_msg"], np.float32),
        np.asarray(inputs["gru_Wi"], np.float32),
        np.asarray(inputs["gru_Wh"], np.float32),
    )
    b_msg = np.asarray(inputs["b_msg"], np.float32)
    assert np.abs(b_msg).max() == 0.0, "nonzero b_msg not supported"

    idx16, S_img, CB = _preprocess_edges(src, dst, etype)

    mask = np.zeros((1, 1024), np.float32)
    for gg in range(GPC):
        mask[0, gg * 64:gg * 64 + 63] = 1.0
    ident = np.eye(128).astype(BF16)

    in_maps = []
    for c in range(NCORES):
        feats = features[c * NPC:(c + 1) * NPC]  # [4096, 128]
        h0 = np.zeros((NPC, OUT), np.float32)
        h0[:, :IN_DIM] = feats
        h08 = np.zeros((128, 2, NPC), np.float32)
        h08[:, 0, :] = h0[:, 0:128].T
        h08[:, 1, :] = h0[:, 128:256].T
        im = {
            "h0T": h0.T.astype(BF16),
            "h0r8": h0.astype(FP8),
            "h08": h08.astype(FP8),
            "xT": feats.T.astype(BF16),
            "Wc": Wc,
            "Wf8": Wf8,
            "Wi8": Wi8,
            "Wh8": Wh8,
            "Bimg": Bimg,
            "Smat": S_img[c],
            "idx": idx16[c],
            "mask": mask,
            "ident": ident,
        }
        in_maps.append(im)
    return wl, bl, CB, in_maps


def kernel(**inputs):
    wl, bl, CB, in_maps = _prepare(inputs)
    nc = build_program(CB, wl, bl)
    res = run_bass_kernel_spmd(nc, in_maps, core_ids=list(range(NCORES)))
    out = np.concatenate([res.results[c]["out"] for c in range(NCORES)])
    return out.astype(np.float32)


# revision 30
# speedup vs baseline: 1.1053x; 1.0104x over previous
"""Devign GGNN model on 8 Trainium2 NeuronCores (Bass/Tile) — v2.

Strategy (data-parallel over dst-node shards, aggregate-first message
passing, fp8 message path):
  - 8 cores, core c owns nodes [c*4096, (c+1)*4096) = graphs [c*16, +16).
  - Node state hT kept transposed on-chip ([256 feat, 4096 nodes] bf16)
    plus an fp8 kc-interleaved copy h8 [128, 2, 4096] for DoubleRow GRU.
  - Per GGNN step:
      1. AllGather the raw fp8 node states: in_cc [4096, 256] fp8 ->
         out_cc [32768, 256] fp8 (8x less wire traffic than gathering the
         per-etype transformed table in bf16).
      2. Edges (sorted by (dst-window, etype), chunked into 128-edge
         slots) are processed per 2-window group: indirect-DMA row gather
         of h[src] (fp8 256B rows, 4 SWDGE queues round-robin), then a
         one-hot scatter matmul agg_t += chunk^T @ S per (window, etype)
         using fp8 DoubleRow (2 chunks per matmul), S resident in SBUF.
      3. Transform after aggregation: aT[:, w] = sum_t W_msg[t]^T @ agg_t
         (fp8 DoubleRow) — 4x fewer transform FLOPs than transforming
         every node by every etype.
      4. GRU: all six gate matmuls as fp8 DoubleRow (contraction 256 in
         one instruction); gate pointwise + state update in bf16/f32.
      5. New h rows for the next AllGather produced by PE transpose +
         fp8 cast per 128-node chunk.
  - Readout (conv1d/maxpool stacks + gated sum) evaluated batched over
    all 16 graphs at full width with tap-shifted matmuls; invalid
    cross-graph columns masked before the final per-graph reduction.

All index/one-hot/weight-layout preprocessing is done on the host at
kernel() time and baked into the compiled program + per-core inputs.
"""
import sys
import numpy as np

for _p in ("/opt/trn_rl_repo",):
    if _p not in sys.path:
        sys.path.insert(0, _p)

import ml_dtypes

import concourse.bass as bass
import concourse.mybir as mybir
import concourse.tile as tile
from concourse import bacc
from concourse.bass_utils import run_bass_kernel_spmd

BF16 = ml_dtypes.bfloat16
FP8 = ml_dtypes.float8_e4m3
F32 = np.float32

NCORES = 8
NN = 32768          # total nodes
IN_DIM = 128
OUT = 256
NT = 4              # edge types
NSTEPS = 8
NGRAPH = 128
NPC = NN // NCORES  # nodes per core = 4096
WIN = 128           # dst window size
NWIN = NPC // WIN   # 32 windows per core
GW = 2              # windows per group / GRU block of 256 nodes
NGRP = NWIN // GW   # 16 groups
GPC = NGRAPH // NCORES  # graphs per core = 16
LG = 256            # nodes per graph
CONCAT = IN_DIM + OUT  # 384
CALLCH = 4          # chunks per dma_gather call (512 rows)
NQ = 4              # SWDGE queues

bf = mybir.dt.bfloat16
f32 = mybir.dt.float32
fp8 = mybir.dt.float8e4
i16 = mybir.dt.int16
AF = mybir.ActivationFunctionType
ALU = mybir.AluOpType
DR = mybir.MatmulPerfMode.DoubleRow


# ---------------------------------------------------------------------------
# chunk layout shared between host packer and device builder
# ---------------------------------------------------------------------------
def _chunk_plan(CB):
    """CB: [NWIN][NT] chunk counts. Returns (boff[w][t] global chunk offset,
    SCBW[w], TOTCH, calls: list over grp of list of (c0, c1, coloff))."""
    CB = [[int(x) for x in row] for row in CB]
    boff = []
    tot = 0
    SCBW = []
    for w in range(NWIN):
        row = []
        for t in range(NT):
            row.append(tot)
            tot += CB[w][t]
        boff.append(row)
        SCBW.append(sum(CB[w]))
    TOTCH = tot
    calls = []
    coloff = 0
    for g in range(NGRP):
        c0 = boff[g * GW][0]
        c1 = c0 + SCBW[g * GW] + SCBW[g * GW + 1]
        grp_calls = []
        c = c0
        while c < c1:
            ce = min(c + CALLCH, c1)
            grp_calls.append((c, ce, coloff))
            coloff += (ce - c) * 128 // 16
            c = ce
        calls.append(grp_calls)
    return boff, SCBW, TOTCH, calls, coloff


# ---------------------------------------------------------------------------
# weight/bias image layout (bf16 conv image)
# ---------------------------------------------------------------------------
class WLayout:
    def __init__(self):
        self.col = 0
        self.off = {}

    def alloc(self, name, width):
        self.off[name] = self.col
        self.col += width
        return self.off[name]


def _make_wlayout():
    wl = WLayout()
    for tap in range(3):
        for kc in range(2):
            for mo in range(2):
                wl.alloc(f"c1_{tap}_{kc}_{mo}", 128)
    for kc in range(2):
        for mo in range(2):
            wl.alloc(f"c2_{kc}_{mo}", 128)
    for tap in range(3):
        for kc in range(3):
            for mo in range(3):
                wl.alloc(f"cc1_{tap}_{kc}_{mo}", 128)
    for kc in range(3):
        for mo in range(3):
            wl.alloc(f"cc2_{kc}_{mo}", 128)
    for kc in range(2):
        wl.alloc(f"wy_{kc}", 1)
    for kc in range(3):
        wl.alloc(f"wz_{kc}", 1)
    return wl


def _make_blayout():
    bl = WLayout()
    for name in ("br", "bz", "big", "bhg"):
        bl.alloc(name, 2)       # [256] as two [128] cols
    bl.alloc("c1b", 2)
    bl.alloc("c2b", 2)
    bl.alloc("cc1b", 3)
    bl.alloc("cc2b", 3)
    bl.alloc("by", 1)
    bl.alloc("bz_", 1)
    return bl


def _pack_weights(wl, conv1_w, conv2_w, convc1_w, convc2_w, wy, wz):
    img = np.zeros((128, wl.col), np.float32)

    def put(name, block):
        o = wl.off[name]
        img[:, o:o + block.shape[1]] = block

    for tap in range(3):
        w_t = conv1_w[:, :, tap].T  # [i, o]
        for kc in range(2):
            for mo in range(2):
                put(f"c1_{tap}_{kc}_{mo}", w_t[kc * 128:(kc + 1) * 128, mo * 128:(mo + 1) * 128])
    w2 = conv2_w[:, :, 0].T
    for kc in range(2):
        for mo in range(2):
            put(f"c2_{kc}_{mo}", w2[kc * 128:(kc + 1) * 128, mo * 128:(mo + 1) * 128])
    for tap in range(3):
        w_t = convc1_w[:, :, tap].T
        for kc in range(3):
            for mo in range(3):
                put(f"cc1_{tap}_{kc}_{mo}", w_t[kc * 128:(kc + 1) * 128, mo * 128:(mo + 1) * 128])
    wc2 = convc2_w[:, :, 0].T
    for kc in range(3):
        for mo in range(3):
            put(f"cc2_{kc}_{mo}", wc2[kc * 128:(kc + 1) * 128, mo * 128:(mo + 1) * 128])
    for kc in range(2):
        put(f"wy_{kc}", wy[kc * 128:(kc + 1) * 128, :])
    for kc in range(3):
        put(f"wz_{kc}", wz[kc * 128:(kc + 1) * 128, :])
    return img.astype(BF16)


def _pack_biases(bl, gru_bi, gru_bh, conv1_b, conv2_b, convc1_b, convc2_b, by, bz_):
    img = np.zeros((128, bl.col), np.float32)

    def put(name, vec, nch):
        o = bl.off[name]
        for c in range(nch):
            img[:, o + c] = vec[c * 128:(c + 1) * 128]

    put("br", gru_bi[0:256] + gru_bh[0:256], 2)
    put("bz", gru_bi[256:512] + gru_bh[256:512], 2)
    put("big", gru_bi[512:768], 2)
    put("bhg", gru_bh[512:768], 2)
    put("c1b", conv1_b, 2)
    put("c2b", conv2_b, 2)
    put("cc1b", convc1_b, 3)
    put("cc2b", convc2_b, 3)
    img[0, bl.off["by"]] = by[0]
    img[0, bl.off["bz_"]] = bz_[0]
    return img


def _pack_fp8_weights(W_msg, gru_Wi, gru_Wh):
    """fp8 DoubleRow lhsT images: [k 128, block, kc 2, m 128] where
    element [k, b, kc, m] = W[kc*128 + k, bcol(b)*128 + m]."""
    wf8 = np.zeros((128, NT * 2, 2, 128), np.float32)
    for t in range(NT):
        for mo in range(2):
            for kc in range(2):
                wf8[:, t * 2 + mo, kc, :] = W_msg[t][kc * 128:(kc + 1) * 128,
                                                     mo * 128:(mo + 1) * 128]
    wi8 = np.zeros((128, 6, 2, 128), np.float32)
    wh8 = np.zeros((128, 6, 2, 128), np.float32)
    for mi in range(6):
        for kc in range(2):
            wi8[:, mi, kc, :] = gru_Wi[kc * 128:(kc + 1) * 128, mi * 128:(mi + 1) * 128]
            wh8[:, mi, kc, :] = gru_Wh[kc * 128:(kc + 1) * 128, mi * 128:(mi + 1) * 128]
    return wf8.astype(FP8), wi8.astype(FP8), wh8.astype(FP8)


# ---------------------------------------------------------------------------
# edge preprocessing: bucket by (core, dst-window, etype), chunk by 128
# ---------------------------------------------------------------------------
def _preprocess_edges(src, dst, etype):
    core = dst // NPC
    w = (dst % NPC) // WIN
    t = etype
    dloc = (dst % WIN).astype(np.int64)
    key = (core * NWIN + w) * NT + t
    order = np.argsort(key, kind="stable")
    cnt = np.bincount(key, minlength=NCORES * NWIN * NT).reshape(NCORES, NWIN, NT)
    CB = np.maximum(1, -(-cnt // 128)).max(axis=0)      # [NWIN, NT]
    boff, SCBW, TOTCH, calls, ncoltot = _chunk_plan(CB)
    starts = np.zeros(NCORES * NWIN * NT + 1, np.int64)
    starts[1:] = np.cumsum(cnt.reshape(-1))
    ar128 = np.arange(128)

    S_img = np.zeros((NCORES, 128, TOTCH, 128), np.float32)
    L_img = np.zeros((NCORES, TOTCH * 128), np.int16)   # slot src ids
    for c in range(NCORES):
        for wdx in range(NWIN):
            for tt in range(NT):
                s0 = starts[(c * NWIN + wdx) * NT + tt]
                s1 = starts[(c * NWIN + wdx) * NT + tt + 1]
                seg = order[s0:s1]
                nch = int(CB[wdx][tt])
                pad = nch * 128 - len(seg)
                li = np.concatenate([src[seg].astype(np.int64),
                                     np.zeros(pad, np.int64)]).astype(np.int16)
                dl = np.concatenate([dloc[seg], np.full(pad, -1, np.int64)])
                b0 = boff[wdx][tt]
                L_img[c][b0 * 128:(b0 + nch) * 128] = li
                dlb = dl.reshape(nch, 128)
                for j in range(nch):
                    valid = dlb[j] >= 0
                    S_img[c][ar128[valid], b0 + j, dlb[j][valid]] = 1.0

    idx16 = np.zeros((NCORES, 128, ncoltot), np.int16)
    for c in range(NCORES):
        for g in range(NGRP):
            for (c0, c1, coloff) in calls[g]:
                flat = L_img[c][c0 * 128:c1 * 128]
                ncol = len(flat) // 16
                wrapped = flat.reshape(ncol, 16).T
                idx16[c][:, coloff:coloff + ncol] = np.tile(wrapped, (8, 1))
    return idx16, S_img.astype(FP8), CB


# ---------------------------------------------------------------------------
# device program
# ---------------------------------------------------------------------------
def build_program(CB, wl, bl, num_devices=NCORES, sim_mode=False, nsteps=NSTEPS):
    boff, SCBW, TOTCH, calls, ncoltot = _chunk_plan(CB)
    NCHG = [SCBW[g * GW] + SCBW[g * GW + 1] for g in range(NGRP)]
    NCHMAX = max(NCHG)
    nc = bacc.Bacc("TRN2", target_bir_lowering=False, debug=False,
                   num_devices=num_devices, num_swdge_queues=NQ)
    h0T_d = nc.dram_tensor("h0T", [OUT, NPC], bf, kind="ExternalInput")
    h0r8_d = nc.dram_tensor("h0r8", [NPC, OUT], fp8, kind="ExternalInput")
    h08_d = nc.dram_tensor("h08", [128, 2, NPC], fp8, kind="ExternalInput")
    xT_d = nc.dram_tensor("xT", [IN_DIM, NPC], bf, kind="ExternalInput")
    Wc_d = nc.dram_tensor("Wc", [128, wl.col], bf, kind="ExternalInput")
    Wf8_d = nc.dram_tensor("Wf8", [128, NT * 2, 2, 128], fp8, kind="ExternalInput")
    Wi8_d = nc.dram_tensor("Wi8", [128, 6, 2, 128], fp8, kind="ExternalInput")
    Wh8_d = nc.dram_tensor("Wh8", [128, 6, 2, 128], fp8, kind="ExternalInput")
    Bimg_d = nc.dram_tensor("Bimg", [128, bl.col], f32, kind="ExternalInput")
    Smat_d = nc.dram_tensor("Smat", [128, TOTCH, 128], fp8, kind="ExternalInput")
    idx_d = nc.dram_tensor("idx", [128, ncoltot], i16, kind="ExternalInput")
    mask_d = nc.dram_tensor("mask", [1, 1024], f32, kind="ExternalInput")
    ident_d = nc.dram_tensor("ident", [128, 128], bf, kind="ExternalInput")
    out_d = nc.dram_tensor("out", [GPC], f32, kind="ExternalOutput")

    with tile.TileContext(nc) as tc:
        with tc.tile_pool(name="persist", bufs=1) as pp, \
             tc.tile_pool(name="dram", bufs=1, space="DRAM") as dpool:
            hT0 = pp.tile([128, NPC], bf)
            hT1 = pp.tile([128, NPC], bf)
            h8 = pp.tile([128, 2, NPC], fp8)
            Wcs = pp.tile([128, wl.col], bf)
            Wf8 = pp.tile([128, NT * 2, 2, 128], fp8)
            Wi8 = pp.tile([128, 6, 2, 128], fp8)
            Wh8 = pp.tile([128, 6, 2, 128], fp8)
            Bsb = pp.tile([128, bl.col], f32)
            ident = pp.tile([128, 128], bf)
            nc.sync.dma_start(hT0[:], h0T_d.ap()[0:128, :])
            nc.sync.dma_start(hT1[:], h0T_d.ap()[128:256, :])
            nc.sync.dma_start(h8[:], h08_d.ap())
            nc.sync.dma_start(Wcs[:], Wc_d.ap())
            nc.sync.dma_start(Wf8[:], Wf8_d.ap())
            nc.sync.dma_start(Wi8[:], Wi8_d.ap())
            nc.sync.dma_start(Wh8[:], Wh8_d.ap())
            nc.sync.dma_start(Bsb[:], Bimg_d.ap())
            nc.sync.dma_start(ident[:], ident_d.ap())
            hT = [hT0, hT1]

            def W(name):
                o = wl.off[name]
                return Wcs[:, o:o + (1 if name[0] == 'w' else 128)]

            def Bias(name, c=0):
                o = bl.off[name] + c
                return Bsb[:, o:o + 1]

            def Bias1(name):
                o = bl.off[name]
                return Bsb[0:1, o:o + 1]

            in_cc0 = dpool.tile([NPC, OUT], fp8, name="in_cc0")
            nc.sync.dma_start(in_cc0[:], h0r8_d.ap())
            in_cc = [dpool.tile([NPC, OUT], fp8, name=f"in_cc{s}")
                     for s in range(1, nsteps)]
            out_cc = [dpool.tile([NN, OUT], fp8, addr_space="Shared",
                                 name=f"out_cc{s}") for s in range(nsteps)]

            # ---------------- GGNN loop ----------------
            with tc.tile_pool(name="sload", bufs=1) as slp, \
                 tc.tile_pool(name="gat", bufs=2) as gap, \
                 tc.tile_pool(name="stage", bufs=2) as stp, \
                 tc.tile_pool(name="gtmp", bufs=2) as gtp, \
                 tc.tile_pool(name="pA", bufs=1, space="PSUM") as pA, \
                 tc.tile_pool(name="pT", bufs=1, space="PSUM") as pT, \
                 tc.tile_pool(name="pG", bufs=1, space="PSUM") as pG, \
                 tc.tile_pool(name="pX", bufs=1, space="PSUM") as pX:
                S_sb = slp.tile([128, TOTCH, 128], fp8)
                idx_sb = slp.tile([128, ncoltot], i16)
                nc.sync.dma_start(S_sb[:], Smat_d.ap())
                nc.sync.dma_start(idx_sb[:], idx_d.ap())

                qi = 0
                for s in range(nsteps):
                    with nc.named_scope(f"s{s}_ag"):
                        ag_in = in_cc0[:] if s == 0 else in_cc[s - 1][:]
                        if sim_mode:
                            nc.sync.dma_start(out_cc[s][0:NPC, :], ag_in)
                        else:
                            nc.gpsimd.collective_compute(
                                "AllGather", ALU.bypass,
                                replica_groups=[list(range(num_devices))],
                                ins=[ag_in.opt()],
                                outs=[out_cc[s].opt()],
                            )
                    ctx_edge = nc.named_scope(f"s{s}_edge")
                    ctx_edge.__enter__()
                    for grp in range(NGRP):
                        gbase = boff[grp * GW][0]
                        gt = gap.tile([128, NCHMAX, OUT], fp8, tag="gt")
                        for (c0, c1, coloff) in calls[grp]:
                            nrow = (c1 - c0) * 128
                            nc.gpsimd.dma_gather(
                                gt[:, c0 - gbase:c1 - gbase, :],
                                out_cc[s][:],
                                idx_sb[:, coloff:coloff + nrow // 16],
                                nrow, nrow, OUT, queue_num=qi)
                            qi = (qi + 1) % NQ
                        aT8 = gtp.tile([128, 2, 256], fp8, tag="aT8")
                        for wi in range(GW):
                            w = grp * GW + wi
                            ps_agg = pA.tile([128, NT, 2, 128], f32, space="PSUM",
                                             tag="agg")
                            for t in range(NT):
                                cb = int(CB[w][t])
                                o = boff[w][t]
                                npair = cb // 2
                                odd = cb % 2
                                for fc in range(2):
                                    for j in range(npair):
                                        nc.tensor.matmul(
                                            ps_agg[:, t, fc, :],
                                            lhsT=gt[:, o - gbase + 2 * j:o - gbase + 2 * j + 2,
                                                    fc * 128:(fc + 1) * 128],
                                            rhs=S_sb[:, o + 2 * j:o + 2 * j + 2, :],
                                            start=(j == 0), stop=(j == npair - 1 and not odd),
                                            perf_mode=DR)
                                    if odd:
                                        nc.tensor.matmul(
                                            ps_agg[:, t, fc, :],
                                            lhsT=gt[:, o - gbase + cb - 1,
                                                    fc * 128:(fc + 1) * 128],
                                            rhs=S_sb[:, o + cb - 1, :],
                                            start=(npair == 0), stop=True)
                            agg8 = [gtp.tile([128, 2, 128], fp8, tag=f"agg8_{t}",
                                             name=f"agg8_{t}")
                                    for t in range(NT)]
                            for t in range(NT):
                                for fc in range(2):
                                    if (t + fc) % 2 == 0:
                                        nc.vector.tensor_copy(agg8[t][:, fc, :],
                                                              ps_agg[:, t, fc, :])
                                    else:
                                        nc.scalar.activation(agg8[t][:, fc, :],
                                                             ps_agg[:, t, fc, :], AF.Copy)
                            ps_aT = pT.tile([128, 2, 128], f32, space="PSUM", tag="aT")
                            for mo in range(2):
                                for t in range(NT):
                                    nc.tensor.matmul(
                                        ps_aT[:, mo, :],
                                        lhsT=Wf8[:, t * 2 + mo, :, :],
                                        rhs=agg8[t][:],
                                        start=(t == 0), stop=(t == NT - 1),
                                        perf_mode=DR)
                            for mo in range(2):
                                if mo == 0:
                                    nc.vector.tensor_copy(
                                        aT8[:, mo, wi * 128:(wi + 1) * 128],
                                        ps_aT[:, mo, :])
                                else:
                                    nc.scalar.activation(
                                        aT8[:, mo, wi * 128:(wi + 1) * 128],
                                        ps_aT[:, mo, :], AF.Copy)
                        # ---- GRU on block nodes [grp*256, +256) ----
                        nb = grp * GW * WIN
                        h8blk = h8[:, :, nb:nb + 256]
                        for fc in range(2):
                            ps = pG.tile([128, 1024], f32, space="PSUM", tag=f"g{fc}")
                            for gi, col in ((0, 0), (1, 256)):
                                mi = gi * 2 + fc
                                nc.tensor.matmul(ps[:, col:col + 256],
                                                 lhsT=Wi8[:, mi, :, :], rhs=aT8[:],
                                                 start=True, stop=False, perf_mode=DR)
                                nc.tensor.matmul(ps[:, col:col + 256],
                                                 lhsT=Wh8[:, mi, :, :], rhs=h8blk,
                                                 start=False, stop=True, perf_mode=DR)
                            mi = 4 + fc
                            nc.tensor.matmul(ps[:, 512:768],
                                             lhsT=Wi8[:, mi, :, :], rhs=aT8[:],
                                             start=True, stop=True, perf_mode=DR)
                            nc.tensor.matmul(ps[:, 768:1024],
                                             lhsT=Wh8[:, mi, :, :], rhs=h8blk,
                                             start=True, stop=True, perf_mode=DR)
                            hslice = hT[fc][:, nb:nb + 256]
                            r = gtp.tile([128, 256], bf, tag="r")
                            z = gtp.tile([128, 256], bf, tag="z")
                            t1 = gtp.tile([128, 256], f32, tag="t1")
                            g = gtp.tile([128, 256], f32, tag="g2")
                            d = gtp.tile([128, 256], f32, tag="d")
                            nc.scalar.activation(r[:], ps[:, 0:256], AF.Sigmoid,
                                                 bias=Bias("br", fc))
                            nc.scalar.activation(z[:], ps[:, 256:512], AF.Sigmoid,
                                                 bias=Bias("bz", fc))
                            nc.vector.scalar_tensor_tensor(
                                t1[:], ps[:, 768:1024], Bias("bhg", fc), r[:],
                                op0=ALU.add, op1=ALU.mult)
                            nc.vector.tensor_add(t1[:], t1[:], ps[:, 512:768])
                            nc.scalar.activation(g[:], t1[:], AF.Tanh,
                                                 bias=Bias("big", fc))
                            nc.vector.tensor_sub(d[:], hslice, g[:])
                            nc.vector.tensor_mul(d[:], z[:], d[:])
                            nc.vector.tensor_add(hslice, g[:], d[:])
                            # refresh fp8 interleaved state
                            nc.scalar.activation(h8[:, fc, nb:nb + 256], hslice,
                                                 AF.Copy)
                        # rows for the next step's AllGather
                        if s < nsteps - 1:
                            for sub in range(GW):
                                n0 = nb + sub * 128
                                rows8 = stp.tile([128, 2, 128], fp8, tag="rows8",
                                                 name="rows8")
                                for kc in range(2):
                                    psT = pX.tile([128, 128], bf, space="PSUM",
                                                  tag="T")
                                    nc.tensor.matmul(psT[:],
                                                     lhsT=hT[kc][:, n0:n0 + 128],
                                                     rhs=ident[:],
                                                     start=True, stop=True,
                                                     is_transpose=True)
                                    if kc == 0:
                                        nc.vector.tensor_copy(rows8[:, kc, :], psT[:])
                                    else:
                                        nc.scalar.activation(rows8[:, kc, :], psT[:],
                                                             AF.Copy)
                                nc.sync.dma_start(in_cc[s][n0:n0 + 128, :], rows8[:])
                    ctx_edge.__exit__(None, None, None)

            # ---------------- readout (batched over 16 graphs) ----------------
            with tc.tile_pool(name="rsb", bufs=1) as rsb, \
                 tc.tile_pool(name="rtmp", bufs=1) as rtp, \
                 tc.tile_pool(name="pR", bufs=2, space="PSUM") as pR, \
                 tc.tile_pool(name="pV", bufs=1, space="PSUM") as pV:
                ctx_ro = nc.named_scope("readout")
                ctx_ro.__enter__()
                res_sb = pp.tile([1, GPC], f32)
                xTb = rsb.tile([128, NPC], bf)
                mask_sb = rsb.tile([1, 1024], f32)
                nc.sync.dma_start(xTb[:], xT_d.ap())
                nc.sync.dma_start(mask_sb[:], mask_d.ap())

                W1 = 4094          # conv1 output cols computed (of 4096)
                P1 = 2046          # pool1 output cols
                P2 = 1023          # pool2 output cols

                # --- Y path ---
                y1 = []
                for mo in range(2):
                    yt = rsb.tile([128, W1], bf, tag=f"w1_{mo}")
                    for ch0 in range(0, W1, 512):
                        cw = min(512, W1 - ch0)
                        psc = pR.tile([128, 512], f32, space="PSUM", tag="conv")
                        k = 0
                        for tap in range(3):
                            for kc in range(2):
                                nc.tensor.matmul(
                                    psc[:, 0:cw],
                                    lhsT=W(f"c1_{tap}_{kc}_{mo}"),
                                    rhs=hT[kc][:, ch0 + tap:ch0 + tap + cw],
                                    start=(k == 0), stop=(k == 5))
                                k += 1
                        nc.scalar.activation(yt[:, ch0:ch0 + cw], psc[:, 0:cw],
                                             AF.Relu, bias=Bias("c1b", mo))
                    y1.append(yt)

                def pool3(src, width, outw, tag):
                    # out[q] = max(src[2q], src[2q+1], src[2q+2]); q < outw
                    pout = rsb.tile([128, outw], bf, tag=tag)
                    tmp = rtp.tile([128, outw], bf, tag=tag + "t")
                    ab = src[:, 0:2 * outw].rearrange("p (n t) -> p n t", t=2)
                    nc.vector.tensor_max(tmp[:], ab[:, :, 0], ab[:, :, 1])
                    a2 = src[:, 2:2 * outw + 2].rearrange("p (n t) -> p n t", t=2)[:, :, 0]
                    nc.vector.tensor_max(pout[:], tmp[:], a2)
                    return pout

                def pool2(src, width, outw, tag):
                    pout = rsb.tile([128, outw], bf, tag=tag)
                    ab = src[:, 0:2 * outw].rearrange("p (n t) -> p n t", t=2)
                    nc.vector.tensor_max(pout[:], ab[:, :, 0], ab[:, :, 1])
                    return pout

                y2 = [pool3(y1[mo], W1, P1, f"p1_{mo}") for mo in range(2)]
                y3 = []
                for mo in range(2):
                    yt = rsb.tile([128, P1], bf, tag=f"c2_{mo}")
                    for ch0 in range(0, P1, 512):
                        cw = min(512, P1 - ch0)
                        psc = pR.tile([128, 512], f32, space="PSUM", tag="conv")
                        for kc in range(2):
                            nc.tensor.matmul(psc[:, 0:cw],
                                             lhsT=W(f"c2_{kc}_{mo}"),
                                             rhs=y2[kc][:, ch0:ch0 + cw],
                                             start=(kc == 0), stop=(kc == 1))
                        nc.scalar.activation(yt[:, ch0:ch0 + cw], psc[:, 0:cw],
                                             AF.Relu, bias=Bias("c2b", mo))
                    y3.append(yt)
                y4 = [pool2(y3[mo], P1, P2, f"p2_{mo}") for mo in range(2)]
                psy = pV.tile([1, 1024], f32, space="PSUM", tag="psy")
                for ch0 in range(0, P2, 512):
                    cw = min(512, P2 - ch0)
                    for kc in range(2):
                        nc.tensor.matmul(psy[:, ch0:ch0 + cw],
                                         lhsT=W(f"wy_{kc}"),
                                         rhs=y4[kc][:, ch0:ch0 + cw],
                                         start=(kc == 0), stop=(kc == 1))
                # --- Z path ---
                cch = [hT[0], hT[1], xTb]
                z1 = []
                for mo in range(3):
                    zt = rsb.tile([128, W1], bf, tag=f"w1_{mo}")
                    for ch0 in range(0, W1, 512):
                        cw = min(512, W1 - ch0)
                        psc = pR.tile([128, 512], f32, space="PSUM", tag="conv")
                        k = 0
                        for tap in range(3):
                            for kc in range(3):
                                nc.tensor.matmul(
                                    psc[:, 0:cw],
                                    lhsT=W(f"cc1_{tap}_{kc}_{mo}"),
                                    rhs=cch[kc][:, ch0 + tap:ch0 + tap + cw],
                                    start=(k == 0), stop=(k == 8))
                                k += 1
                        nc.scalar.activation(zt[:, ch0:ch0 + cw], psc[:, 0:cw],
                                             AF.Relu, bias=Bias("cc1b", mo))
                    z1.append(zt)
                z2 = [pool3(z1[mo], W1, P1, f"p1_{mo}") for mo in range(3)]
                z3 = []
                for mo in range(3):
                    zt = rsb.tile([128, P1], bf, tag=f"c2_{mo}")
                    for ch0 in range(0, P1, 512):
                        cw = min(512, P1 - ch0)
                        psc = pR.tile([128, 512], f32, space="PSUM", tag="conv")
                        for kc in range(3):
                            nc.tensor.matmul(psc[:, 0:cw],
                                             lhsT=W(f"cc2_{kc}_{mo}"),
                                             rhs=z2[kc][:, ch0:ch0 + cw],
                                             start=(kc == 0), stop=(kc == 2))
                        nc.scalar.activation(zt[:, ch0:ch0 + cw], psc[:, 0:cw],
                                             AF.Relu, bias=Bias("cc2b", mo))
                    z3.append(zt)
                z4 = [pool2(z3[mo], P1, P2, f"p2_{mo}") for mo in range(3)]
                psz = pV.tile([1, 1024], f32, space="PSUM", tag="psz")
                for ch0 in range(0, P2, 512):
                    cw = min(512, P2 - ch0)
                    for kc in range(3):
                        nc.tensor.matmul(psz[:, ch0:ch0 + cw],
                                         lhsT=W(f"wz_{kc}"),
                                         rhs=z4[kc][:, ch0:ch0 + cw],
                                         start=(kc == 0), stop=(kc == 2))
                # --- combine ---
                ty = rsb.tile([1, 1024], f32, tag="ty")
                tz = rsb.tile([1, 1024], f32, tag="tz")
                pr = rsb.tile([1, 1024], f32, tag="pr")
                sm = rsb.tile([1, GPC], f32, tag="sm")
                nc.vector.memset(ty[:], 0.0)
                nc.vector.memset(tz[:], 0.0)
                nc.vector.tensor_scalar_add(ty[:, 0:P2], psy[:, 0:P2], Bias1("by"))
                nc.vector.tensor_scalar_add(tz[:, 0:P2], psz[:, 0:P2], Bias1("bz_"))
                nc.vector.tensor_mul(pr[:], ty[:], tz[:])
                nc.vector.tensor_mul(pr[:], pr[:], mask_sb[:])
                nc.vector.tensor_reduce(
                    sm[:], pr[:].rearrange("p (g c) -> p g c", c=64),
                    axis=mybir.AxisListType.X, op=ALU.add)
                nc.scalar.activation(res_sb[0:1, :], sm[:],
                                     AF.Sigmoid, scale=1.0 / 63.0)
                nc.sync.dma_start(out_d.ap(), res_sb[0:1, :])
                ctx_ro.__exit__(None, None, None)
    nc.finalize()
    return nc


# ---------------------------------------------------------------------------
# host entry
# ---------------------------------------------------------------------------
def _prepare(inputs):
    features = np.asarray(inputs["features"], np.float32)
    src = np.asarray(inputs["src"]).astype(np.int64)
    dst = np.asarray(inputs["dst"]).astype(np.int64)
    etype = np.asarray(inputs["etype"]).astype(np.int64)
    wl = _make_wlayout()
    bl = _make_blayout()
    Wc = _pack_weights(
        wl,
        np.asarray(inputs["conv1_w"], np.float32),
        np.asarray(inputs["conv2_w"], np.float32),
        np.asarray(inputs["convc1_w"], np.float32),
        np.asarray(inputs["convc2_w"], np.float32),
        np.asarray(inputs["wy"], np.float32),
        np.asarray(inputs["wz"], np.float32),
    )
    Bimg = _pack_biases(
        bl,
        np.asarray(inputs["gru_bi"], np.float32),
        np.asarray(inputs["gru_bh"], np.float32),
        np.asarray(inputs["conv1_b"], np.float32),
        np.asarray(inputs["conv2_b"], np.float32),
        np.asarray(inputs["convc1_b"], np.float32),
        np.asarray(inputs["convc2_b"], np.float32),
        np.asarray(inputs["by"], np.float32),
        np.asarray(inputs["bz"], np.float32),
    )
    Wf8, Wi8, Wh8 = _pack_fp8_weights(
        np.asarray(inputs["W_msg"], np.float32),
        np.asarray(inputs["gru_Wi"], np.float32),
        np.asarray(inputs["gru_Wh"], np.float32),
    )
    b_msg = np.asarray(inputs["b_msg"], np.float32)
    assert np.abs(b_msg).max() == 0.0, "nonzero b_msg not supported"

    idx16, S_img, CB = _preprocess_edges(src, dst, etype)

    mask = np.zeros((1, 1024), np.float32)
    for gg in range(GPC):
        mask[0, gg * 64:gg * 64 + 63] = 1.0
    ident = np.eye(128).astype(BF16)

    in_maps = []
    for c in range(NCORES):
        feats = features[c * NPC:(c + 1) * NPC]  # [4096, 128]
        h0 = np.zeros((NPC, OUT), np.float32)
        h0[:, :IN_DIM] = feats
        h08 = np.zeros((128, 2, NPC), np.float32)
        h08[:, 0, :] = h0[:, 0:128].T
        h08[:, 1, :] = h0[:, 128:256].T
        im = {
            "h0T": h0.T.astype(BF16),
            "h0r8": h0.astype(FP8),
            "h08": h08.astype(FP8),
            "xT": feats.T.astype(BF16),
            "Wc": Wc,
            "Wf8": Wf8,
            "Wi8": Wi8,
            "Wh8": Wh8,
            "Bimg": Bimg,
            "Smat": S_img[c],
            "idx": idx16[c],
            "mask": mask,
            "ident": ident,
        }
        in_maps.append(im)
    return wl, bl, CB, in_maps


def kernel(**inputs):
    wl, bl, CB, in_maps = _prepare(inputs)
    nc = build_program(CB, wl, bl)
    res = run_bass_kernel_spmd(nc, in_maps, core_ids=list(range(NCORES)))
    out = np.concatenate([res.results[c]["out"] for c in range(NCORES)])
    return out.astype(np.float32)


# revision 31
# speedup vs baseline: 1.1275x; 1.0201x over previous
"""Devign GGNN model on 8 Trainium2 NeuronCores (Bass/Tile) — v2.

Strategy (data-parallel over dst-node shards, aggregate-first message
passing, fp8 message path):
  - 8 cores, core c owns nodes [c*4096, (c+1)*4096) = graphs [c*16, +16).
  - Node state hT kept transposed on-chip ([256 feat, 4096 nodes] bf16)
    plus an fp8 kc-interleaved copy h8 [128, 2, 4096] for DoubleRow GRU.
  - Per GGNN step:
      1. AllGather the raw fp8 node states: in_cc [4096, 256] fp8 ->
         out_cc [32768, 256] fp8 (8x less wire traffic than gathering the
         per-etype transformed table in bf16).
      2. Edges (sorted by (dst-window, etype), chunked into 128-edge
         slots) are processed per 2-window group: indirect-DMA row gather
         of h[src] (fp8 256B rows, 4 SWDGE queues round-robin), then a
         one-hot scatter matmul agg_t += chunk^T @ S per (window, etype)
         using fp8 DoubleRow (2 chunks per matmul), S resident in SBUF.
      3. Transform after aggregation: aT[:, w] = sum_t W_msg[t]^T @ agg_t
         (fp8 DoubleRow) — 4x fewer transform FLOPs than transforming
         every node by every etype.
      4. GRU: all six gate matmuls as fp8 DoubleRow (contraction 256 in
         one instruction); gate pointwise + state update in bf16/f32.
      5. New h rows for the next AllGather produced by PE transpose +
         fp8 cast per 128-node chunk.
  - Readout (conv1d/maxpool stacks + gated sum) evaluated batched over
    all 16 graphs at full width with tap-shifted matmuls; invalid
    cross-graph columns masked before the final per-graph reduction.

All index/one-hot/weight-layout preprocessing is done on the host at
kernel() time and baked into the compiled program + per-core inputs.
"""
import sys
import numpy as np

for _p in ("/opt/trn_rl_repo",):
    if _p not in sys.path:
        sys.path.insert(0, _p)

import ml_dtypes

import concourse.bass as bass
import concourse.mybir as mybir
import concourse.tile as tile
from concourse import bacc
from concourse.bass_utils import run_bass_kernel_spmd

BF16 = ml_dtypes.bfloat16
FP8 = ml_dtypes.float8_e4m3
F32 = np.float32

NCORES = 8
NN = 32768          # total nodes
IN_DIM = 128
OUT = 256
NT = 4              # edge types
NSTEPS = 8
NGRAPH = 128
NPC = NN // NCORES  # nodes per core = 4096
WIN = 128           # dst window size
NWIN = NPC // WIN   # 32 windows per core
GW = 2              # windows per group / GRU block of 256 nodes
NGRP = NWIN // GW   # 16 groups
GPC = NGRAPH // NCORES  # graphs per core = 16
LG = 256            # nodes per graph
CONCAT = IN_DIM + OUT  # 384
CALLCH = 2          # chunks per dma_gather call (256 rows)
NQ = 4              # SWDGE queues

bf = mybir.dt.bfloat16
f32 = mybir.dt.float32
fp8 = mybir.dt.float8e4
i16 = mybir.dt.int16
AF = mybir.ActivationFunctionType
ALU = mybir.AluOpType
DR = mybir.MatmulPerfMode.DoubleRow


# ---------------------------------------------------------------------------
# chunk layout shared between host packer and device builder
# ---------------------------------------------------------------------------
def _chunk_plan(CB):
    """CB: [NWIN][NT] chunk counts. Returns (boff[w][t] global chunk offset,
    SCBW[w], TOTCH, calls: list over grp of list of (c0, c1, coloff))."""
    CB = [[int(x) for x in row] for row in CB]
    boff = []
    tot = 0
    SCBW = []
    for w in range(NWIN):
        row = []
        for t in range(NT):
            row.append(tot)
            tot += CB[w][t]
        boff.append(row)
        SCBW.append(sum(CB[w]))
    TOTCH = tot
    calls = []
    coloff = 0
    for g in range(NGRP):
        c0 = boff[g * GW][0]
        c1 = c0 + SCBW[g * GW] + SCBW[g * GW + 1]
        grp_calls = []
        c = c0
        while c < c1:
            ce = min(c + CALLCH, c1)
            grp_calls.append((c, ce, coloff))
            coloff += (ce - c) * 128 // 16
            c = ce
        calls.append(grp_calls)
    return boff, SCBW, TOTCH, calls, coloff


# ---------------------------------------------------------------------------
# weight/bias image layout (bf16 conv image)
# ---------------------------------------------------------------------------
class WLayout:
    def __init__(self):
        self.col = 0
        self.off = {}

    def alloc(self, name, width):
        self.off[name] = self.col
        self.col += width
        return self.off[name]


def _make_wlayout():
    wl = WLayout()
    for tap in range(3):
        for kc in range(2):
            for mo in range(2):
                wl.alloc(f"c1_{tap}_{kc}_{mo}", 128)
    for kc in range(2):
        for mo in range(2):
            wl.alloc(f"c2_{kc}_{mo}", 128)
    for tap in range(3):
        for kc in range(3):
            for mo in range(3):
                wl.alloc(f"cc1_{tap}_{kc}_{mo}", 128)
    for kc in range(3):
        for mo in range(3):
            wl.alloc(f"cc2_{kc}_{mo}", 128)
    for kc in range(2):
        wl.alloc(f"wy_{kc}", 1)
    for kc in range(3):
        wl.alloc(f"wz_{kc}", 1)
    return wl


def _make_blayout():
    bl = WLayout()
    for name in ("br", "bz", "big", "bhg"):
        bl.alloc(name, 2)       # [256] as two [128] cols
    bl.alloc("c1b", 2)
    bl.alloc("c2b", 2)
    bl.alloc("cc1b", 3)
    bl.alloc("cc2b", 3)
    bl.alloc("by", 1)
    bl.alloc("bz_", 1)
    return bl


def _pack_weights(wl, conv1_w, conv2_w, convc1_w, convc2_w, wy, wz):
    img = np.zeros((128, wl.col), np.float32)

    def put(name, block):
        o = wl.off[name]
        img[:, o:o + block.shape[1]] = block

    for tap in range(3):
        w_t = conv1_w[:, :, tap].T  # [i, o]
        for kc in range(2):
            for mo in range(2):
                put(f"c1_{tap}_{kc}_{mo}", w_t[kc * 128:(kc + 1) * 128, mo * 128:(mo + 1) * 128])
    w2 = conv2_w[:, :, 0].T
    for kc in range(2):
        for mo in range(2):
            put(f"c2_{kc}_{mo}", w2[kc * 128:(kc + 1) * 128, mo * 128:(mo + 1) * 128])
    for tap in range(3):
        w_t = convc1_w[:, :, tap].T
        for kc in range(3):
            for mo in range(3):
                put(f"cc1_{tap}_{kc}_{mo}", w_t[kc * 128:(kc + 1) * 128, mo * 128:(mo + 1) * 128])
    wc2 = convc2_w[:, :, 0].T
    for kc in range(3):
        for mo in range(3):
            put(f"cc2_{kc}_{mo}", wc2[kc * 128:(kc + 1) * 128, mo * 128:(mo + 1) * 128])
    for kc in range(2):
        put(f"wy_{kc}", wy[kc * 128:(kc + 1) * 128, :])
    for kc in range(3):
        put(f"wz_{kc}", wz[kc * 128:(kc + 1) * 128, :])
    return img.astype(BF16)


def _pack_biases(bl, gru_bi, gru_bh, conv1_b, conv2_b, convc1_b, convc2_b, by, bz_):
    img = np.zeros((128, bl.col), np.float32)

    def put(name, vec, nch):
        o = bl.off[name]
        for c in range(nch):
            img[:, o + c] = vec[c * 128:(c + 1) * 128]

    put("br", gru_bi[0:256] + gru_bh[0:256], 2)
    put("bz", gru_bi[256:512] + gru_bh[256:512], 2)
    put("big", gru_bi[512:768], 2)
    put("bhg", gru_bh[512:768], 2)
    put("c1b", conv1_b, 2)
    put("c2b", conv2_b, 2)
    put("cc1b", convc1_b, 3)
    put("cc2b", convc2_b, 3)
    img[0, bl.off["by"]] = by[0]
    img[0, bl.off["bz_"]] = bz_[0]
    return img


def _pack_fp8_weights(W_msg, gru_Wi, gru_Wh):
    """fp8 DoubleRow lhsT images: [k 128, block, kc 2, m 128] where
    element [k, b, kc, m] = W[kc*128 + k, bcol(b)*128 + m]."""
    wf8 = np.zeros((128, NT * 2, 2, 128), np.float32)
    for t in range(NT):
        for mo in range(2):
            for kc in range(2):
                wf8[:, t * 2 + mo, kc, :] = W_msg[t][kc * 128:(kc + 1) * 128,
                                                     mo * 128:(mo + 1) * 128]
    wi8 = np.zeros((128, 6, 2, 128), np.float32)
    wh8 = np.zeros((128, 6, 2, 128), np.float32)
    for mi in range(6):
        for kc in range(2):
            wi8[:, mi, kc, :] = gru_Wi[kc * 128:(kc + 1) * 128, mi * 128:(mi + 1) * 128]
            wh8[:, mi, kc, :] = gru_Wh[kc * 128:(kc + 1) * 128, mi * 128:(mi + 1) * 128]
    return wf8.astype(FP8), wi8.astype(FP8), wh8.astype(FP8)


# ---------------------------------------------------------------------------
# edge preprocessing: bucket by (core, dst-window, etype), chunk by 128
# ---------------------------------------------------------------------------
def _preprocess_edges(src, dst, etype):
    core = dst // NPC
    w = (dst % NPC) // WIN
    t = etype
    dloc = (dst % WIN).astype(np.int64)
    key = (core * NWIN + w) * NT + t
    order = np.argsort(key, kind="stable")
    cnt = np.bincount(key, minlength=NCORES * NWIN * NT).reshape(NCORES, NWIN, NT)
    CB = np.maximum(1, -(-cnt // 128)).max(axis=0)      # [NWIN, NT]
    boff, SCBW, TOTCH, calls, ncoltot = _chunk_plan(CB)
    starts = np.zeros(NCORES * NWIN * NT + 1, np.int64)
    starts[1:] = np.cumsum(cnt.reshape(-1))
    ar128 = np.arange(128)

    S_img = np.zeros((NCORES, 128, TOTCH, 128), np.float32)
    L_img = np.zeros((NCORES, TOTCH * 128), np.int16)   # slot src ids
    for c in range(NCORES):
        for wdx in range(NWIN):
            for tt in range(NT):
                s0 = starts[(c * NWIN + wdx) * NT + tt]
                s1 = starts[(c * NWIN + wdx) * NT + tt + 1]
                seg = order[s0:s1]
                nch = int(CB[wdx][tt])
                pad = nch * 128 - len(seg)
                li = np.concatenate([src[seg].astype(np.int64),
                                     np.zeros(pad, np.int64)]).astype(np.int16)
                dl = np.concatenate([dloc[seg], np.full(pad, -1, np.int64)])
                b0 = boff[wdx][tt]
                L_img[c][b0 * 128:(b0 + nch) * 128] = li
                dlb = dl.reshape(nch, 128)
                for j in range(nch):
                    valid = dlb[j] >= 0
                    S_img[c][ar128[valid], b0 + j, dlb[j][valid]] = 1.0

    idx16 = np.zeros((NCORES, 128, ncoltot), np.int16)
    for c in range(NCORES):
        for g in range(NGRP):
            for (c0, c1, coloff) in calls[g]:
                flat = L_img[c][c0 * 128:c1 * 128]
                ncol = len(flat) // 16
                wrapped = flat.reshape(ncol, 16).T
                idx16[c][:, coloff:coloff + ncol] = np.tile(wrapped, (8, 1))
    return idx16, S_img.astype(FP8), CB


# ---------------------------------------------------------------------------
# device program
# ---------------------------------------------------------------------------
def build_program(CB, wl, bl, num_devices=NCORES, sim_mode=False, nsteps=NSTEPS):
    boff, SCBW, TOTCH, calls, ncoltot = _chunk_plan(CB)
    NCHG = [SCBW[g * GW] + SCBW[g * GW + 1] for g in range(NGRP)]
    NCHMAX = max(NCHG)
    nc = bacc.Bacc("TRN2", target_bir_lowering=False, debug=False,
                   num_devices=num_devices, num_swdge_queues=NQ)
    h0T_d = nc.dram_tensor("h0T", [OUT, NPC], bf, kind="ExternalInput")
    h0r8_d = nc.dram_tensor("h0r8", [NPC, OUT], fp8, kind="ExternalInput")
    h08_d = nc.dram_tensor("h08", [128, 2, NPC], fp8, kind="ExternalInput")
    xT_d = nc.dram_tensor("xT", [IN_DIM, NPC], bf, kind="ExternalInput")
    Wc_d = nc.dram_tensor("Wc", [128, wl.col], bf, kind="ExternalInput")
    Wf8_d = nc.dram_tensor("Wf8", [128, NT * 2, 2, 128], fp8, kind="ExternalInput")
    Wi8_d = nc.dram_tensor("Wi8", [128, 6, 2, 128], fp8, kind="ExternalInput")
    Wh8_d = nc.dram_tensor("Wh8", [128, 6, 2, 128], fp8, kind="ExternalInput")
    Bimg_d = nc.dram_tensor("Bimg", [128, bl.col], f32, kind="ExternalInput")
    Smat_d = nc.dram_tensor("Smat", [128, TOTCH, 128], fp8, kind="ExternalInput")
    idx_d = nc.dram_tensor("idx", [128, ncoltot], i16, kind="ExternalInput")
    mask_d = nc.dram_tensor("mask", [1, 1024], f32, kind="ExternalInput")
    ident_d = nc.dram_tensor("ident", [128, 128], bf, kind="ExternalInput")
    out_d = nc.dram_tensor("out", [GPC], f32, kind="ExternalOutput")

    with tile.TileContext(nc) as tc:
        with tc.tile_pool(name="persist", bufs=1) as pp, \
             tc.tile_pool(name="dram", bufs=1, space="DRAM") as dpool:
            hT0 = pp.tile([128, NPC], bf)
            hT1 = pp.tile([128, NPC], bf)
            h8 = pp.tile([128, 2, NPC], fp8)
            Wcs = pp.tile([128, wl.col], bf)
            Wf8 = pp.tile([128, NT * 2, 2, 128], fp8)
            Wi8 = pp.tile([128, 6, 2, 128], fp8)
            Wh8 = pp.tile([128, 6, 2, 128], fp8)
            Bsb = pp.tile([128, bl.col], f32)
            ident = pp.tile([128, 128], bf)
            nc.sync.dma_start(hT0[:], h0T_d.ap()[0:128, :])
            nc.sync.dma_start(hT1[:], h0T_d.ap()[128:256, :])
            nc.sync.dma_start(h8[:], h08_d.ap())
            nc.sync.dma_start(Wcs[:], Wc_d.ap())
            nc.sync.dma_start(Wf8[:], Wf8_d.ap())
            nc.sync.dma_start(Wi8[:], Wi8_d.ap())
            nc.sync.dma_start(Wh8[:], Wh8_d.ap())
            nc.sync.dma_start(Bsb[:], Bimg_d.ap())
            nc.sync.dma_start(ident[:], ident_d.ap())
            hT = [hT0, hT1]

            def W(name):
                o = wl.off[name]
                return Wcs[:, o:o + (1 if name[0] == 'w' else 128)]

            def Bias(name, c=0):
                o = bl.off[name] + c
                return Bsb[:, o:o + 1]

            def Bias1(name):
                o = bl.off[name]
                return Bsb[0:1, o:o + 1]

            in_cc0 = dpool.tile([NPC, OUT], fp8, name="in_cc0")
            nc.sync.dma_start(in_cc0[:], h0r8_d.ap())
            in_cc = [dpool.tile([NPC, OUT], fp8, name=f"in_cc{s}")
                     for s in range(1, nsteps)]
            out_cc = [dpool.tile([NN, OUT], fp8, addr_space="Shared",
                                 name=f"out_cc{s}") for s in range(nsteps)]

            # ---------------- GGNN loop ----------------
            with tc.tile_pool(name="sload", bufs=1) as slp, \
                 tc.tile_pool(name="gat", bufs=2) as gap, \
                 tc.tile_pool(name="stage", bufs=2) as stp, \
                 tc.tile_pool(name="gtmp", bufs=2) as gtp, \
                 tc.tile_pool(name="pA", bufs=1, space="PSUM") as pA, \
                 tc.tile_pool(name="pT", bufs=1, space="PSUM") as pT, \
                 tc.tile_pool(name="pG", bufs=1, space="PSUM") as pG, \
                 tc.tile_pool(name="pX", bufs=1, space="PSUM") as pX:
                S_sb = slp.tile([128, TOTCH, 128], fp8)
                idx_sb = slp.tile([128, ncoltot], i16)
                nc.sync.dma_start(S_sb[:], Smat_d.ap())
                nc.sync.dma_start(idx_sb[:], idx_d.ap())

                qi = 0
                for s in range(nsteps):
                    with nc.named_scope(f"s{s}_ag"):
                        ag_in = in_cc0[:] if s == 0 else in_cc[s - 1][:]
                        if sim_mode:
                            nc.sync.dma_start(out_cc[s][0:NPC, :], ag_in)
                        else:
                            nc.gpsimd.collective_compute(
                                "AllGather", ALU.bypass,
                                replica_groups=[list(range(num_devices))],
                                ins=[ag_in.opt()],
                                outs=[out_cc[s].opt()],
                            )
                    ctx_edge = nc.named_scope(f"s{s}_edge")
                    ctx_edge.__enter__()
                    for grp in range(NGRP):
                        gbase = boff[grp * GW][0]
                        gt = gap.tile([128, NCHMAX, OUT], fp8, tag="gt")
                        for (c0, c1, coloff) in calls[grp]:
                            nrow = (c1 - c0) * 128
                            nc.gpsimd.dma_gather(
                                gt[:, c0 - gbase:c1 - gbase, :],
                                out_cc[s][:],
                                idx_sb[:, coloff:coloff + nrow // 16],
                                nrow, nrow, OUT, queue_num=qi)
                            qi = (qi + 1) % NQ
                        aT8 = gtp.tile([128, 2, 256], fp8, tag="aT8")
                        for wi in range(GW):
                            w = grp * GW + wi
                            ps_agg = pA.tile([128, NT, 2, 128], f32, space="PSUM",
                                             tag="agg")
                            for t in range(NT):
                                cb = int(CB[w][t])
                                o = boff[w][t]
                                npair = cb // 2
                                odd = cb % 2
                                for fc in range(2):
                                    for j in range(npair):
                                        nc.tensor.matmul(
                                            ps_agg[:, t, fc, :],
                                            lhsT=gt[:, o - gbase + 2 * j:o - gbase + 2 * j + 2,
                                                    fc * 128:(fc + 1) * 128],
                                            rhs=S_sb[:, o + 2 * j:o + 2 * j + 2, :],
                                            start=(j == 0), stop=(j == npair - 1 and not odd),
                                            perf_mode=DR)
                                    if odd:
                                        nc.tensor.matmul(
                                            ps_agg[:, t, fc, :],
                                            lhsT=gt[:, o - gbase + cb - 1,
                                                    fc * 128:(fc + 1) * 128],
                                            rhs=S_sb[:, o + cb - 1, :],
                                            start=(npair == 0), stop=True)
                            agg8 = [gtp.tile([128, 2, 128], fp8, tag=f"agg8_{t}",
                                             name=f"agg8_{t}")
                                    for t in range(NT)]
                            for t in range(NT):
                                for fc in range(2):
                                    if (t + fc) % 2 == 0:
                                        nc.vector.tensor_copy(agg8[t][:, fc, :],
                                                              ps_agg[:, t, fc, :])
                                    else:
                                        nc.scalar.activation(agg8[t][:, fc, :],
                                                             ps_agg[:, t, fc, :], AF.Copy)
                            ps_aT = pT.tile([128, 2, 128], f32, space="PSUM", tag="aT")
                            for mo in range(2):
                                for t in range(NT):
                                    nc.tensor.matmul(
                                        ps_aT[:, mo, :],
                                        lhsT=Wf8[:, t * 2 + mo, :, :],
                                        rhs=agg8[t][:],
                                        start=(t == 0), stop=(t == NT - 1),
                                        perf_mode=DR)
                            for mo in range(2):
                                if mo == 0:
                                    nc.vector.tensor_copy(
                                        aT8[:, mo, wi * 128:(wi + 1) * 128],
                                        ps_aT[:, mo, :])
                                else:
                                    nc.scalar.activation(
                                        aT8[:, mo, wi * 128:(wi + 1) * 128],
                                        ps_aT[:, mo, :], AF.Copy)
                        # ---- GRU on block nodes [grp*256, +256) ----
                        nb = grp * GW * WIN
                        h8blk = h8[:, :, nb:nb + 256]
                        for fc in range(2):
                            ps = pG.tile([128, 1024], f32, space="PSUM", tag=f"g{fc}")
                            for gi, col in ((0, 0), (1, 256)):
                                mi = gi * 2 + fc
                                nc.tensor.matmul(ps[:, col:col + 256],
                                                 lhsT=Wi8[:, mi, :, :], rhs=aT8[:],
                                                 start=True, stop=False, perf_mode=DR)
                                nc.tensor.matmul(ps[:, col:col + 256],
                                                 lhsT=Wh8[:, mi, :, :], rhs=h8blk,
                                                 start=False, stop=True, perf_mode=DR)
                            mi = 4 + fc
                            nc.tensor.matmul(ps[:, 512:768],
                                             lhsT=Wi8[:, mi, :, :], rhs=aT8[:],
                                             start=True, stop=True, perf_mode=DR)
                            nc.tensor.matmul(ps[:, 768:1024],
                                             lhsT=Wh8[:, mi, :, :], rhs=h8blk,
                                             start=True, stop=True, perf_mode=DR)
                            hslice = hT[fc][:, nb:nb + 256]
                            r = gtp.tile([128, 256], bf, tag="r")
                            z = gtp.tile([128, 256], bf, tag="z")
                            t1 = gtp.tile([128, 256], f32, tag="t1")
                            g = gtp.tile([128, 256], f32, tag="g2")
                            d = gtp.tile([128, 256], f32, tag="d")
                            nc.scalar.activation(r[:], ps[:, 0:256], AF.Sigmoid,
                                                 bias=Bias("br", fc))
                            nc.scalar.activation(z[:], ps[:, 256:512], AF.Sigmoid,
                                                 bias=Bias("bz", fc))
                            nc.vector.scalar_tensor_tensor(
                                t1[:], ps[:, 768:1024], Bias("bhg", fc), r[:],
                                op0=ALU.add, op1=ALU.mult)
                            nc.vector.tensor_add(t1[:], t1[:], ps[:, 512:768])
                            nc.scalar.activation(g[:], t1[:], AF.Tanh,
                                                 bias=Bias("big", fc))
                            nc.vector.tensor_sub(d[:], hslice, g[:])
                            nc.vector.tensor_mul(d[:], z[:], d[:])
                            nc.vector.tensor_add(hslice, g[:], d[:])
                            # refresh fp8 interleaved state
                            nc.scalar.activation(h8[:, fc, nb:nb + 256], hslice,
                                                 AF.Copy)
                        # rows for the next step's AllGather
                        if s < nsteps - 1:
                            for sub in range(GW):
                                n0 = nb + sub * 128
                                rows8 = stp.tile([128, 2, 128], fp8, tag="rows8",
                                                 name="rows8")
                                for kc in range(2):
                                    psT = pX.tile([128, 128], bf, space="PSUM",
                                                  tag="T")
                                    nc.tensor.matmul(psT[:],
                                                     lhsT=hT[kc][:, n0:n0 + 128],
                                                     rhs=ident[:],
                                                     start=True, stop=True,
                                                     is_transpose=True)
                                    if kc == 0:
                                        nc.vector.tensor_copy(rows8[:, kc, :], psT[:])
                                    else:
                                        nc.scalar.activation(rows8[:, kc, :], psT[:],
                                                             AF.Copy)
                                nc.sync.dma_start(in_cc[s][n0:n0 + 128, :], rows8[:])
                    ctx_edge.__exit__(None, None, None)

            # ---------------- readout (batched over 16 graphs) ----------------
            with tc.tile_pool(name="rsb", bufs=1) as rsb, \
                 tc.tile_pool(name="rtmp", bufs=1) as rtp, \
                 tc.tile_pool(name="pR", bufs=2, space="PSUM") as pR, \
                 tc.tile_pool(name="pV", bufs=1, space="PSUM") as pV:
                ctx_ro = nc.named_scope("readout")
                ctx_ro.__enter__()
                res_sb = pp.tile([1, GPC], f32)
                xTb = rsb.tile([128, NPC], bf)
                mask_sb = rsb.tile([1, 1024], f32)
                nc.sync.dma_start(xTb[:], xT_d.ap())
                nc.sync.dma_start(mask_sb[:], mask_d.ap())

                W1 = 4094          # conv1 output cols computed (of 4096)
                P1 = 2046          # pool1 output cols
                P2 = 1023          # pool2 output cols

                # --- Y path ---
                y1 = []
                for mo in range(2):
                    yt = rsb.tile([128, W1], bf, tag=f"w1_{mo}")
                    for ch0 in range(0, W1, 512):
                        cw = min(512, W1 - ch0)
                        psc = pR.tile([128, 512], f32, space="PSUM", tag="conv")
                        k = 0
                        for tap in range(3):
                            for kc in range(2):
                                nc.tensor.matmul(
                                    psc[:, 0:cw],
                                    lhsT=W(f"c1_{tap}_{kc}_{mo}"),
                                    rhs=hT[kc][:, ch0 + tap:ch0 + tap + cw],
                                    start=(k == 0), stop=(k == 5))
                                k += 1
                        nc.scalar.activation(yt[:, ch0:ch0 + cw], psc[:, 0:cw],
                                             AF.Relu, bias=Bias("c1b", mo))
                    y1.append(yt)

                def pool3(src, width, outw, tag):
                    # out[q] = max(src[2q], src[2q+1], src[2q+2]); q < outw
                    pout = rsb.tile([128, outw], bf, tag=tag)
                    tmp = rtp.tile([128, outw], bf, tag=tag + "t")
                    ab = src[:, 0:2 * outw].rearrange("p (n t) -> p n t", t=2)
                    nc.vector.tensor_max(tmp[:], ab[:, :, 0], ab[:, :, 1])
                    a2 = src[:, 2:2 * outw + 2].rearrange("p (n t) -> p n t", t=2)[:, :, 0]
                    nc.vector.tensor_max(pout[:], tmp[:], a2)
                    return pout

                def pool2(src, width, outw, tag):
                    pout = rsb.tile([128, outw], bf, tag=tag)
                    ab = src[:, 0:2 * outw].rearrange("p (n t) -> p n t", t=2)
                    nc.vector.tensor_max(pout[:], ab[:, :, 0], ab[:, :, 1])
                    return pout

                y2 = [pool3(y1[mo], W1, P1, f"p1_{mo}") for mo in range(2)]
                y3 = []
                for mo in range(2):
                    yt = rsb.tile([128, P1], bf, tag=f"c2_{mo}")
                    for ch0 in range(0, P1, 512):
                        cw = min(512, P1 - ch0)
                        psc = pR.tile([128, 512], f32, space="PSUM", tag="conv")
                        for kc in range(2):
                            nc.tensor.matmul(psc[:, 0:cw],
                                             lhsT=W(f"c2_{kc}_{mo}"),
                                             rhs=y2[kc][:, ch0:ch0 + cw],
                                             start=(kc == 0), stop=(kc == 1))
                        nc.scalar.activation(yt[:, ch0:ch0 + cw], psc[:, 0:cw],
                                             AF.Relu, bias=Bias("c2b", mo))
                    y3.append(yt)
                y4 = [pool2(y3[mo], P1, P2, f"p2_{mo}") for mo in range(2)]
                psy = pV.tile([1, 1024], f32, space="PSUM", tag="psy")
                for ch0 in range(0, P2, 512):
                    cw = min(512, P2 - ch0)
                    for kc in range(2):
                        nc.tensor.matmul(psy[:, ch0:ch0 + cw],
                                         lhsT=W(f"wy_{kc}"),
                                         rhs=y4[kc][:, ch0:ch0 + cw],
                                         start=(kc == 0), stop=(kc == 1))
                # --- Z path ---
                cch = [hT[0], hT[1], xTb]
                z1 = []
                for mo in range(3):
                    zt = rsb.tile([128, W1], bf, tag=f"w1_{mo}")
                    for ch0 in range(0, W1, 512):
                        cw = min(512, W1 - ch0)
                        psc = pR.tile([128, 512], f32, space="PSUM", tag="conv")
                        k = 0
                        for tap in range(3):
                            for kc in range(3):
                                nc.tensor.matmul(
                                    psc[:, 0:cw],
                                    lhsT=W(f"cc1_{tap}_{kc}_{mo}"),
                                    rhs=cch[kc][:, ch0 + tap:ch0 + tap + cw],
                                    start=(k == 0), stop=(k == 8))
                                k += 1
                        nc.scalar.activation(zt[:, ch0:ch0 + cw], psc[:, 0:cw],
                                             AF.Relu, bias=Bias("cc1b", mo))
                    z1.append(zt)
                z2 = [pool3(z1[mo], W1, P1, f"p1_{mo}") for mo in range(3)]
                z3 = []
                for mo in range(3):
                    zt = rsb.tile([128, P1], bf, tag=f"c2_{mo}")
                    for ch0 in range(0, P1, 512):
                        cw = min(512, P1 - ch0)
                        psc = pR.tile([128, 512], f32, space="PSUM", tag="conv")
                        for kc in range(3):
                            nc.tensor.matmul(psc[:, 0:cw],
                                             lhsT=W(f"cc2_{kc}_{mo}"),
                                             rhs=z2[kc][:, ch0:ch0 + cw],
                                             start=(kc == 0), stop=(kc == 2))
                        nc.scalar.activation(zt[:, ch0:ch0 + cw], psc[:, 0:cw],
                                             AF.Relu, bias=Bias("cc2b", mo))
                    z3.append(zt)
                z4 = [pool2(z3[mo], P1, P2, f"p2_{mo}") for mo in range(3)]
                psz = pV.tile([1, 1024], f32, space="PSUM", tag="psz")
                for ch0 in range(0, P2, 512):
                    cw = min(512, P2 - ch0)
                    for kc in range(3):
                        nc.tensor.matmul(psz[:, ch0:ch0 + cw],
                                         lhsT=W(f"wz_{kc}"),
                                         rhs=z4[kc][:, ch0:ch0 + cw],
                                         start=(kc == 0), stop=(kc == 2))
                # --- combine ---
                ty = rsb.tile([1, 1024], f32, tag="ty")
                tz = rsb.tile([1, 1024], f32, tag="tz")
                pr = rsb.tile([1, 1024], f32, tag="pr")
                sm = rsb.tile([1, GPC], f32, tag="sm")
                nc.vector.memset(ty[:], 0.0)
                nc.vector.memset(tz[:], 0.0)
                nc.vector.tensor_scalar_add(ty[:, 0:P2], psy[:, 0:P2], Bias1("by"))
                nc.vector.tensor_scalar_add(tz[:, 0:P2], psz[:, 0:P2], Bias1("bz_"))
                nc.vector.tensor_mul(pr[:], ty[:], tz[:])
                nc.vector.tensor_mul(pr[:], pr[:], mask_sb[:])
                nc.vector.tensor_reduce(
                    sm[:], pr[:].rearrange("p (g c) -> p g c", c=64),
                    axis=mybir.AxisListType.X, op=ALU.add)
                nc.scalar.activation(res_sb[0:1, :], sm[:],
                                     AF.Sigmoid, scale=1.0 / 63.0)
                nc.sync.dma_start(out_d.ap(), res_sb[0:1, :])
                ctx_ro.__exit__(None, None, None)
    nc.finalize()
    return nc


# ---------------------------------------------------------------------------
# host entry
# ---------------------------------------------------------------------------
def _prepare(inputs):
    features = np.asarray(inputs["features"], np.float32)
    src = np.asarray(inputs["src"]).astype(np.int64)
    dst = np.asarray(inputs["dst"]).astype(np.int64)
    etype = np.asarray(inputs["etype"]).astype(np.int64)
    wl = _make_wlayout()
    bl = _make_blayout()
    Wc = _pack_weights(
        wl,
        np.asarray(inputs["conv1_w"], np.float32),
        np.asarray(inputs["conv2_w"], np.float32),
        np.asarray(inputs["convc1_w"], np.float32),
        np.asarray(inputs["convc2_w"], np.float32),
        np.asarray(inputs["wy"], np.float32),
        np.asarray(inputs["wz"], np.float32),
    )
    Bimg = _pack_biases(
        bl,
        np.asarray(inputs["gru_bi"], np.float32),
        np.asarray(inputs["gru_bh"], np.float32),
        np.asarray(inputs["conv1_b"], np.float32),
        np.asarray(inputs["conv2_b"], np.float32),
        np.asarray(inputs["convc1_b"], np.float32),
        np.asarray(inputs["convc2_b"], np.float32),
        np.asarray(inputs["by"], np.float32),
        np.asarray(inputs["bz"], np.float32),
    )
    Wf8, Wi8, Wh8 = _pack_fp8_weights(
        np.asarray(inputs["W_msg"], np.float32),
        np.asarray(inputs["gru_Wi"], np.float32),
        np.asarray(inputs["gru_Wh"], np.float32),
    )
    b_msg = np.asarray(inputs["b_msg"], np.float32)
    assert np.abs(b_msg).max() == 0.0, "nonzero b_msg not supported"

    idx16, S_img, CB = _preprocess_edges(src, dst, etype)

    mask = np.zeros((1, 1024), np.float32)
    for gg in range(GPC):
        mask[0, gg * 64:gg * 64 + 63] = 1.0
    ident = np.eye(128).astype(BF16)

    in_maps = []
    for c in range(NCORES):
        feats = features[c * NPC:(c + 1) * NPC]  # [4096, 128]
        h0 = np.zeros((NPC, OUT), np.float32)
        h0[:, :IN_DIM] = feats
        h08 = np.zeros((128, 2, NPC), np.float32)
        h08[:, 0, :] = h0[:, 0:128].T
        h08[:, 1, :] = h0[:, 128:256].T
        im = {
            "h0T": h0.T.astype(BF16),
            "h0r8": h0.astype(FP8),
            "h08": h08.astype(FP8),
            "xT": feats.T.astype(BF16),
            "Wc": Wc,
            "Wf8": Wf8,
            "Wi8": Wi8,
            "Wh8": Wh8,
            "Bimg": Bimg,
            "Smat": S_img[c],
            "idx": idx16[c],
            "mask": mask,
            "ident": ident,
        }
        in_maps.append(im)
    return wl, bl, CB, in_maps


def kernel(**inputs):
    wl, bl, CB, in_maps = _prepare(inputs)
    nc = build_program(CB, wl, bl)
    res = run_bass_kernel_spmd(nc, in_maps, core_ids=list(range(NCORES)))
    out = np.concatenate([res.results[c]["out"] for c in range(NCORES)])
    return out.astype(np.float32)
